# revision 9
# baseline (speedup 1.0000x reference)
"""M3GNet interaction kernel for 8 Trainium2 NeuronCores.

Device computes the dense per-edge radial-basis MLP activations and the
per-triplet angular MLP activations; edges and triplets are sharded 8 ways
(graph/data parallel). Host does the index-based gathers/segment sums and
the small channel-mixing matmuls.

Device-side structure (per core):
- Edge radial basis exploits Gaussian locality: each edge only sees a
  32-center window of the 64 RBF centers (3 overlapping window classes,
  edges bucketed by distance on host, class resolved via per-supertile
  stationary weights streamed as data). The exp argument
  -gamma*(d-c)^2 + ln(env) comes from one K=32 block-diagonal matmul per
  2048-edge supertile (4 chunks of 512 packed into 128 partitions), with
  hi/lo bf16 operand splitting for fp32-grade accuracy at full PE rate
  (the per-center -gamma*c'^2 bias is folded in as hi/lo constant rows).
- Triplet path: one K=12 block-diagonal matmul per 1024 triplets.
- softplus: native Softplus activation, or a staged Exp->Ln(1+x) pipeline
  batched by activation function to avoid ACT table-set thrash.
Outputs stream back as bf16.
"""
import numpy as np
import ml_dtypes

import concourse.bacc as bacc
import concourse.bass as bass
import concourse.mybir as mybir
from concourse.tile import TileContext
from concourse import bass_utils

BF16 = ml_dtypes.bfloat16

N_NODES = 20000
N_EDGES = 640000
N_TRIP = 1000000
C = 128
E = 64
CUTOFF = 5.0
LOG2 = float(np.log(2.0))
NCORES = 8
DC = CUTOFF / (E - 1)                       # center spacing
GAMMA = 1.0 / (2.0 * (CUTOFF / E) ** 2)
W0S = (0, 16, 32)                           # window starts per class
CB0, CB1 = 21.5, 37.5                       # class boundaries in bin units

ST_E = 2048       # edges per supertile (4 chunks of 512)
MT_T = 1024       # triplets per matmul tile (2 chunks of 512)

USE_SOFTPLUS = False     # no softplus ACT table in this toolchain

_CACHED = {}


def _hilo(x):
    x = np.asarray(x, np.float32)
    hi = x.astype(BF16)
    lo = (x - hi.astype(np.float32)).astype(BF16)
    return hi, lo


def _build(nst_pc, nmt_pc, use_softplus=USE_SOFTPLUS):
    key = (nst_pc, nmt_pc, use_softplus)
    if key in _CACHED:
        return _CACHED[key]
    nc = bacc.Bacc('TRN2', target_bir_lowering=False, debug=False)
    f32 = mybir.dt.float32
    bf16 = mybir.dt.bfloat16
    AF = mybir.ActivationFunctionType

    erows = nc.dram_tensor('erows', [32, nst_pc * 512], bf16, kind='ExternalInput')
    eaw = nc.dram_tensor('eaw', [nst_pc, 32, 128], bf16, kind='ExternalInput')
    ew2 = nc.dram_tensor('ew2', [nst_pc, 128, 128], bf16, kind='ExternalInput')
    trows = nc.dram_tensor('trows', [12, nmt_pc * 512], bf16, kind='ExternalInput')
    tw3 = nc.dram_tensor('tw3', [12, 128], bf16, kind='ExternalInput')

    sT = nc.dram_tensor('sT', [128, nst_pc * 1024], bf16, kind='ExternalOutput')
    uT = nc.dram_tensor('uT', [128, nmt_pc * 512], bf16, kind='ExternalOutput')

    n_eg = (nst_pc + 3) // 4      # edge stage-1 groups of 4 supertiles
    n_bt = (nst_pc + 1) // 2      # edge stage-2 psB tiles (2 supertiles each)
    n_ct = (nmt_pc + 3) // 4      # triplet psC tiles (4 mm tiles each)

    with TileContext(nc) as tc:
        with tc.tile_pool(name='rbe_w', bufs=1) as rbw:
            rbe = rbw.tile([128, nst_pc * 512], bf16, tag='rbe')

            # ---------- edge stage 1: arg matmuls + Exp -> rbe_wide ----------
            with (
                tc.tile_pool(name='e1_in', bufs=3) as e1i,
                tc.tile_pool(name='e1_w', bufs=3) as e1w,
                tc.tile_pool(name='e1_ps', bufs=2, space='PSUM') as ps1,
            ):
                for g in range(n_eg):
                    s0 = g * 4
                    ns = min(4, nst_pc - s0)
                    w = ns * 512
                    psA = ps1.tile([128, 2048], f32, tag='psA')
                    for j in range(ns):
                        s = s0 + j
                        rows = e1i.tile([32, 512], bf16, tag='erows')
                        nc.sync.dma_start(rows[:],
                                          erows[:, s * 512:(s + 1) * 512])
                        awt = e1w.tile([32, 128], bf16, tag='aw')
                        nc.sync.dma_start(awt[:], bass.AP(eaw, s * 32 * 128,
                                                          [[128, 32], [1, 128]]))
                        nc.tensor.matmul(psA[:, j * 512:(j + 1) * 512],
                                         awt[:], rows[:])
                    nc.scalar.activation(rbe[:, s0 * 512:s0 * 512 + w],
                                         psA[:, 0:w], AF.Exp)

            # ---------- edge stage 2: p1 matmuls + softplus + out ----------
            with (
                tc.tile_pool(name='e2_w', bufs=3) as e2w,
                tc.tile_pool(name='e2_sb', bufs=2) as e2s,
                tc.tile_pool(name='e2_se', bufs=1) as e2e,
                tc.tile_pool(name='e2_ps', bufs=2, space='PSUM') as ps2,
            ):
                sexps = []
                for b in range(n_bt):
                    s0 = b * 2
                    ns = min(2, nst_pc - s0)
                    w = ns * 1024
                    psB = ps2.tile([128, 2048], f32, tag='psB')
                    for j in range(ns):
                        s = s0 + j
                        w2t = e2w.tile([128, 128], bf16, tag='w2')
                        nc.sync.dma_start(w2t[:], bass.AP(ew2, s * 128 * 128,
                                                          [[128, 128], [1, 128]]))
                        rb = rbe[:, s * 512:(s + 1) * 512]
                        nc.tensor.matmul(psB[:, j * 1024:j * 1024 + 512],
                                         w2t[0:64, :], rb[0:64, :])
                        nc.tensor.matmul(psB[:, j * 1024 + 512:j * 1024 + 1024],
                                         w2t[64:128, :], rb[64:128, :])
                    if use_softplus:
                        sout = e2s.tile([128, 2048], bf16, tag='sout')
                        nc.scalar.activation(sout[:, 0:w], psB[:, 0:w],
                                             AF.Softplus)
                        nc.sync.dma_start(sT[:, s0 * 1024:s0 * 1024 + w],
                                          sout[:, 0:w])
                    else:
                        sexp = e2e.tile([128, 2048], bf16, tag=f'sexp{b}')
                        nc.scalar.activation(sexp[:, 0:w], psB[:, 0:w], AF.Exp)
                        sexps.append((sexp, s0, w))
                if not use_softplus:
                    for sexp, s0, w in sexps:
                        sout = e2s.tile([128, 2048], bf16, tag='sout')
                        nc.scalar.activation(sout[:, 0:w], sexp[:, 0:w],
                                             AF.Ln, bias=1.0)
                        nc.sync.dma_start(sT[:, s0 * 1024:s0 * 1024 + w],
                                          sout[:, 0:w])

        # ---------- triplet phase ----------
        with (
            tc.tile_pool(name='t_w', bufs=1) as twp,
            tc.tile_pool(name='t_in', bufs=3) as tin,
            tc.tile_pool(name='t_sb', bufs=2) as tsb,
            tc.tile_pool(name='t_ue', bufs=1) as tue,
            tc.tile_pool(name='t_ps', bufs=2, space='PSUM') as tps,
        ):
            w3t = twp.tile([12, 128], bf16, tag='w3')
            nc.sync.dma_start(w3t[:], tw3[:])
            uexp = None
            if not use_softplus:
                uexp = tue.tile([128, nmt_pc * 512], bf16, tag='uexp')
            for b in range(n_ct):
                m0 = b * 4
                nm = min(4, nmt_pc - m0)
                w = nm * 512
                rows = tin.tile([12, 2048], bf16, tag='trows')
                nc.sync.dma_start(rows[:, 0:w],
                                  trows[:, m0 * 512:(m0 + nm) * 512])
                psC = tps.tile([128, 2048], f32, tag='psC')
                for j in range(nm):
                    nc.tensor.matmul(psC[:, j * 512:(j + 1) * 512],
                                     w3t[:], rows[:, j * 512:(j + 1) * 512])
                if use_softplus:
                    uout = tsb.tile([128, 2048], bf16, tag='uout')
                    nc.scalar.activation(uout[:, 0:w], psC[:, 0:w], AF.Softplus)
                    nc.sync.dma_start(uT[:, m0 * 512:(m0 + nm) * 512],
                                      uout[:, 0:w])
                else:
                    nc.scalar.activation(uexp[:, m0 * 512:m0 * 512 + w],
                                         psC[:, 0:w], AF.Exp)
            if not use_softplus:
                ncols = nmt_pc * 512
                step = 4096
                for c0 in range(0, ncols, step):
                    w = min(step, ncols - c0)
                    uout = tsb.tile([128, step], bf16, tag='uoutl')
                    nc.scalar.activation(uout[:, 0:w], uexp[:, c0:c0 + w],
                                         AF.Ln, bias=1.0)
                    nc.sync.dma_start(uT[:, c0:c0 + w], uout[:, 0:w])

    nc.compile()
    _CACHED[key] = nc
    return nc


def _segsum(vals, idx, nseg):
    order = np.argsort(idx, kind='stable')
    sv = vals[order]
    si = idx[order]
    counts = np.bincount(si, minlength=nseg)
    out = np.zeros((nseg, vals.shape[1]), np.float32)
    nz = np.flatnonzero(counts)
    if nz.size:
        starts = np.concatenate([[0], np.cumsum(counts)])[nz]
        out[nz] = np.add.reduceat(sv, starts, axis=0)
    return out


def kernel(features, neighbour_distances, neighbour_list, triplet_idxs,
           angles, r_ij, r_ik, W_pre, W2b1, W2b2, W3b1, W3b2, W_post):
    d_all = np.asarray(neighbour_distances, np.float32)
    nl = np.asarray(neighbour_list)
    t1 = np.asarray(triplet_idxs)[:, 1]
    W2b1 = np.asarray(W2b1, np.float32)
    W2b2 = np.asarray(W2b2, np.float32)
    W3b1 = np.asarray(W3b1, np.float32)
    W3b2 = np.asarray(W3b2, np.float32)
    centers = np.linspace(0.0, CUTOFF, E, dtype=np.float32)

    # ---------------- edge host prep ----------------
    keep = d_all < CUTOFF
    kept_idx = np.flatnonzero(keep)
    d = d_all[kept_idx]
    b = d / DC
    cls = np.where(b < CB0, 0, np.where(b < CB1, 1, 2)).astype(np.int32)
    order = np.argsort(cls, kind='stable')
    kept_sorted = kept_idx[order]
    d_s = d[order]
    ncls = np.bincount(cls[order], minlength=3)

    nst_cls = [(int(n) + ST_E - 1) // ST_E for n in ncls]
    nst_tot = sum(nst_cls)
    nst_tot_pad = ((nst_tot + NCORES - 1) // NCORES) * NCORES
    nst_pc = nst_tot_pad // NCORES
    n_epad = nst_tot_pad * ST_E

    dP = np.zeros(n_epad, np.float32)
    zz = np.full(n_epad, -30.0, np.float32)
    st_cls = np.zeros(nst_tot_pad, np.int32)
    src_pos = np.full(n_epad, -1, np.int64)
    off_e = off_p = st_i = 0
    for c in range(3):
        n = int(ncls[c])
        shift = (W0S[c] + 15.5) * DC
        dseg = d_s[off_e:off_e + n] - shift
        env = 0.5 * (1.0 + np.cos(np.pi * d_s[off_e:off_e + n] / CUTOFF))
        dP[off_p:off_p + n] = dseg
        zz[off_p:off_p + n] = (np.log(np.maximum(env, 1e-35))
                               - GAMMA * dseg * dseg)
        src_pos[off_p:off_p + n] = off_e + np.arange(n)
        st_cls[st_i:st_i + nst_cls[c]] = c
        off_e += n
        off_p += nst_cls[c] * ST_E
        st_i += nst_cls[c]

    d_hi, d_lo = _hilo(dP)
    z_hi, z_lo = _hilo(zz)
    ones = np.ones(n_epad, BF16)
    zero = np.zeros(n_epad, BF16)
    # rows per chunk: [d_hi, d_lo, d_hi, z_hi, z_lo, 1, 1, 0]
    rows8 = np.stack([d_hi, d_lo, d_hi, z_hi, z_lo, ones, ones, zero])
    rows32 = (rows8.reshape(8, nst_tot_pad, 4, 512)
              .transpose(2, 0, 1, 3).reshape(32, nst_tot_pad * 512))

    aw_cls = np.zeros((3, 32, 128), BF16)
    w2_cls = np.zeros((3, 128, 128), BF16)
    for c in range(3):
        w0 = W0S[c]
        cp = centers[w0:w0 + 32] - (w0 + 15.5) * DC
        a_hi, a_lo = _hilo(2.0 * GAMMA * cp)
        b_hi, b_lo = _hilo(-GAMMA * cp * cp)
        one32 = np.ones(32, BF16)
        zero32 = np.zeros(32, BF16)
        blk = np.stack([a_hi, a_hi, a_lo, one32, one32, b_hi, b_lo, zero32])
        for j in range(4):
            aw_cls[c, 8 * j:8 * j + 8, 32 * j:32 * j + 32] = blk
        wwin = W2b1[w0:w0 + 32, :].astype(BF16)
        w2_cls[c, 0:32, 0:64] = wwin
        w2_cls[c, 32:64, 64:128] = wwin
        w2_cls[c, 64:96, 0:64] = wwin
        w2_cls[c, 96:128, 64:128] = wwin

    # ---------------- triplet host prep ----------------
    nmt_tot = (N_TRIP + MT_T - 1) // MT_T
    nmt_tot = ((nmt_tot + NCORES - 1) // NCORES) * NCORES
    ntp = nmt_tot * MT_T
    nmt_pc = nmt_tot // NCORES

    rij = np.zeros(ntp, np.float32)
    rik = np.zeros(ntp, np.float32)
    cosa = np.zeros(ntp, np.float32)
    rij[:N_TRIP] = np.asarray(r_ij, np.float32)
    rik[:N_TRIP] = np.asarray(r_ik, np.float32)
    cosa[:N_TRIP] = np.cos(np.asarray(angles, np.float32))
    rij_h, rij_l = _hilo(rij)
    rik_h, rik_l = _hilo(rik)
    cos_h, cos_l = _hilo(cosa)
    rows6 = np.stack([rij_h, rij_l, rik_h, rik_l, cos_h, cos_l])
    rows12 = (rows6.reshape(6, nmt_tot, 2, 512)
              .transpose(2, 0, 1, 3).reshape(12, nmt_tot * 512))

    w3rows = W3b1.astype(BF16)
    w3dup = np.stack([w3rows[0], w3rows[0], w3rows[1], w3rows[1],
                      w3rows[2], w3rows[2]])
    tw3_np = np.zeros((12, 128), BF16)
    tw3_np[0:6, 0:64] = w3dup
    tw3_np[6:12, 64:128] = w3dup

    # ---------------- build + run ----------------
    nc = _build(nst_pc, nmt_pc)
    in_maps = []
    for k in range(NCORES):
        es = slice(k * nst_pc * 512, (k + 1) * nst_pc * 512)
        sts = slice(k * nst_pc, (k + 1) * nst_pc)
        ts = slice(k * nmt_pc * 512, (k + 1) * nmt_pc * 512)
        ccls = st_cls[sts]
        in_maps.append({
            'erows': np.ascontiguousarray(rows32[:, es]),
            'eaw': np.ascontiguousarray(aw_cls[ccls]),
            'ew2': np.ascontiguousarray(w2_cls[ccls]),
            'trows': np.ascontiguousarray(rows12[:, ts]),
            'tw3': tw3_np,
        })
    res = bass_utils.run_bass_kernel_spmd(nc, in_maps, core_ids=list(range(NCORES)))
    kernel.last_results = res

    sT = np.concatenate([np.asarray(r['sT'], BF16) for r in res.results],
                        axis=1).astype(np.float32)
    uT = np.concatenate([np.asarray(r['uT'], BF16) for r in res.results],
                        axis=1).astype(np.float32)

    s_pad = (sT.reshape(2, 64, nst_tot_pad, 2, 512)
             .transpose(1, 2, 3, 0, 4).reshape(64, n_epad))
    u_pad = (uT.reshape(2, 64, nmt_tot, 512)
             .transpose(1, 2, 0, 3).reshape(64, ntp))

    # ---------------- host combine ----------------
    h = np.asarray(features, np.float32) @ np.asarray(W_pre, np.float32)

    valid = src_pos >= 0
    s_sorted = np.empty((int(ncls.sum()), 64), np.float32)
    s_sorted[src_pos[valid]] = s_pad[:, valid].T
    m_kept = s_sorted @ W2b2
    m_kept += (-LOG2) * W2b2.sum(axis=0)
    nl0_k = nl[0][kept_sorted]
    nl1_k = nl[1][kept_sorted]
    two_body = h[nl1_k] * m_kept
    agg = _segsum(two_body, nl0_k, N_NODES)

    u = u_pad[:, :N_TRIP].T
    U3 = _segsum(u, t1, N_NODES)
    U3 -= LOG2 * np.bincount(t1, minlength=N_NODES)[:, None].astype(np.float32)
    em = h[:N_NODES] * (U3 @ W3b2)
    agg += _segsum(em, nl[0][:N_NODES], N_NODES)

    return (agg @ np.asarray(W_post, np.float32)).astype(np.float32)


# revision 10
# speedup vs baseline: 1.0201x; 1.0201x over previous
"""M3GNet interaction kernel for 8 Trainium2 NeuronCores.

Device computes the dense per-edge radial-basis MLP activations and the
per-triplet angular MLP activations; edges and triplets are sharded 8 ways
(graph/data parallel). Host does the index-based gathers/segment sums and
the small channel-mixing matmuls.

Device-side structure (per core):
- Edge radial basis exploits Gaussian locality: each edge only sees a
  32-center window of the 64 RBF centers (3 overlapping window classes,
  edges bucketed by distance on host, class resolved via per-supertile
  stationary weights streamed as data). The exp argument
  -gamma*(d-c)^2 + ln(env) comes from one K=32 block-diagonal matmul per
  2048-edge supertile (4 chunks of 512 packed into 128 partitions), with
  hi/lo bf16 operand splitting for fp32-grade accuracy at full PE rate
  (the per-center -gamma*c'^2 bias is folded in as hi/lo constant rows).
- Triplet path: one K=12 block-diagonal matmul per 1024 triplets.
- softplus: native Softplus activation, or a staged Exp->Ln(1+x) pipeline
  batched by activation function to avoid ACT table-set thrash.
Outputs stream back as bf16.
"""
import numpy as np
import ml_dtypes

import concourse.bacc as bacc
import concourse.bass as bass
import concourse.mybir as mybir
from concourse.tile import TileContext
from concourse import bass_utils

BF16 = ml_dtypes.bfloat16

N_NODES = 20000
N_EDGES = 640000
N_TRIP = 1000000
C = 128
E = 64
CUTOFF = 5.0
LOG2 = float(np.log(2.0))
NCORES = 8
DC = CUTOFF / (E - 1)                       # center spacing
GAMMA = 1.0 / (2.0 * (CUTOFF / E) ** 2)
W0S = (0, 16, 32)                           # window starts per class
CB0, CB1 = 21.5, 37.5                       # class boundaries in bin units

ST_E = 2048       # edges per supertile (4 chunks of 512)
MT_T = 1024       # triplets per matmul tile (2 chunks of 512)

USE_SOFTPLUS = False     # no softplus ACT table in this toolchain

_CACHED = {}


def _hilo(x):
    x = np.asarray(x, np.float32)
    hi = x.astype(BF16)
    lo = (x - hi.astype(np.float32)).astype(BF16)
    return hi, lo


def _build(nst_pc, nmt_pc, use_softplus=USE_SOFTPLUS):
    key = (nst_pc, nmt_pc, use_softplus)
    if key in _CACHED:
        return _CACHED[key]
    nc = bacc.Bacc('TRN2', target_bir_lowering=False, debug=False)
    f32 = mybir.dt.float32
    bf16 = mybir.dt.bfloat16
    AF = mybir.ActivationFunctionType

    erows = nc.dram_tensor('erows', [32, nst_pc * 512], bf16, kind='ExternalInput')
    eaw = nc.dram_tensor('eaw', [nst_pc, 32, 128], bf16, kind='ExternalInput')
    ew2 = nc.dram_tensor('ew2', [nst_pc, 128, 128], bf16, kind='ExternalInput')
    trows = nc.dram_tensor('trows', [12, nmt_pc * 512], bf16, kind='ExternalInput')
    tw3 = nc.dram_tensor('tw3', [12, 128], bf16, kind='ExternalInput')

    sT = nc.dram_tensor('sT', [128, nst_pc * 1024], bf16, kind='ExternalOutput')
    uT = nc.dram_tensor('uT', [128, nmt_pc * 512], bf16, kind='ExternalOutput')

    n_eg = (nst_pc + 3) // 4      # edge stage-1 groups of 4 supertiles
    n_bt = (nst_pc + 1) // 2      # edge stage-2 psB tiles (2 supertiles each)
    n_ct = (nmt_pc + 3) // 4      # triplet psC tiles (4 mm tiles each)

    with TileContext(nc) as tc:
        with tc.tile_pool(name='rbe_w', bufs=1) as rbw:
            rbe = rbw.tile([128, nst_pc * 512], bf16, tag='rbe')

            # ---------- edge stage 1: arg matmuls + Exp -> rbe_wide ----------
            with (
                tc.tile_pool(name='e1_in', bufs=3) as e1i,
                tc.tile_pool(name='e1_w', bufs=3) as e1w,
                tc.tile_pool(name='e1_ps', bufs=2, space='PSUM') as ps1,
            ):
                for g in range(n_eg):
                    s0 = g * 4
                    ns = min(4, nst_pc - s0)
                    w = ns * 512
                    psA = ps1.tile([128, 2048], f32, tag='psA')
                    for j in range(ns):
                        s = s0 + j
                        rows = e1i.tile([32, 512], bf16, tag='erows')
                        nc.sync.dma_start(rows[:],
                                          erows[:, s * 512:(s + 1) * 512])
                        awt = e1w.tile([32, 128], bf16, tag='aw')
                        nc.sync.dma_start(awt[:], bass.AP(eaw, s * 32 * 128,
                                                          [[128, 32], [1, 128]]))
                        nc.tensor.matmul(psA[:, j * 512:(j + 1) * 512],
                                         awt[:], rows[:])
                    nc.scalar.activation(rbe[:, s0 * 512:s0 * 512 + w],
                                         psA[:, 0:w], AF.Exp)

            # ---------- edge stage 2: p1 matmuls + softplus + out ----------
            with (
                tc.tile_pool(name='e2_w', bufs=3) as e2w,
                tc.tile_pool(name='e2_sb', bufs=2) as e2s,
                tc.tile_pool(name='e2_se', bufs=1) as e2e,
                tc.tile_pool(name='e2_ps', bufs=2, space='PSUM') as ps2,
            ):
                sexp = None
                if not use_softplus:
                    sexp = e2e.tile([128, nst_pc * 1024], bf16, tag='sexp')
                for b in range(n_bt):
                    s0 = b * 2
                    ns = min(2, nst_pc - s0)
                    w = ns * 1024
                    psB = ps2.tile([128, 2048], f32, tag='psB')
                    for j in range(ns):
                        s = s0 + j
                        w2t = e2w.tile([128, 128], bf16, tag='w2')
                        nc.sync.dma_start(w2t[:], bass.AP(ew2, s * 128 * 128,
                                                          [[128, 128], [1, 128]]))
                        rb = rbe[:, s * 512:(s + 1) * 512]
                        nc.tensor.matmul(psB[:, j * 1024:j * 1024 + 512],
                                         w2t[0:64, :], rb[0:64, :])
                        nc.tensor.matmul(psB[:, j * 1024 + 512:j * 1024 + 1024],
                                         w2t[64:128, :], rb[64:128, :])
                    if use_softplus:
                        sout = e2s.tile([128, 2048], bf16, tag='sout')
                        nc.scalar.activation(sout[:, 0:w], psB[:, 0:w],
                                             AF.Softplus)
                        nc.sync.dma_start(sT[:, s0 * 1024:s0 * 1024 + w],
                                          sout[:, 0:w])
                    else:
                        nc.scalar.activation(sexp[:, s0 * 1024:s0 * 1024 + w],
                                             psB[:, 0:w], AF.Exp)
                if not use_softplus:
                    ncols = nst_pc * 1024
                    step = 4096
                    for c0 in range(0, ncols, step):
                        w = min(step, ncols - c0)
                        sout = e2s.tile([128, step], bf16, tag='sout')
                        nc.scalar.activation(sout[:, 0:w], sexp[:, c0:c0 + w],
                                             AF.Ln, bias=1.0)
                        nc.sync.dma_start(sT[:, c0:c0 + w], sout[:, 0:w])

        # ---------- triplet phase ----------
        with (
            tc.tile_pool(name='t_w', bufs=1) as twp,
            tc.tile_pool(name='t_in', bufs=3) as tin,
            tc.tile_pool(name='t_sb', bufs=2) as tsb,
            tc.tile_pool(name='t_ue', bufs=1) as tue,
            tc.tile_pool(name='t_ps', bufs=2, space='PSUM') as tps,
        ):
            w3t = twp.tile([12, 128], bf16, tag='w3')
            nc.sync.dma_start(w3t[:], tw3[:])
            uexp = None
            if not use_softplus:
                uexp = tue.tile([128, nmt_pc * 512], bf16, tag='uexp')
            for b in range(n_ct):
                m0 = b * 4
                nm = min(4, nmt_pc - m0)
                w = nm * 512
                rows = tin.tile([12, 2048], bf16, tag='trows')
                nc.sync.dma_start(rows[:, 0:w],
                                  trows[:, m0 * 512:(m0 + nm) * 512])
                psC = tps.tile([128, 2048], f32, tag='psC')
                for j in range(nm):
                    nc.tensor.matmul(psC[:, j * 512:(j + 1) * 512],
                                     w3t[:], rows[:, j * 512:(j + 1) * 512])
                if use_softplus:
                    uout = tsb.tile([128, 2048], bf16, tag='uout')
                    nc.scalar.activation(uout[:, 0:w], psC[:, 0:w], AF.Softplus)
                    nc.sync.dma_start(uT[:, m0 * 512:(m0 + nm) * 512],
                                      uout[:, 0:w])
                else:
                    nc.scalar.activation(uexp[:, m0 * 512:m0 * 512 + w],
                                         psC[:, 0:w], AF.Exp)
            if not use_softplus:
                ncols = nmt_pc * 512
                step = 4096
                for c0 in range(0, ncols, step):
                    w = min(step, ncols - c0)
                    uout = tsb.tile([128, step], bf16, tag='uoutl')
                    nc.scalar.activation(uout[:, 0:w], uexp[:, c0:c0 + w],
                                         AF.Ln, bias=1.0)
                    nc.sync.dma_start(uT[:, c0:c0 + w], uout[:, 0:w])

    nc.compile()
    _CACHED[key] = nc
    return nc


def _segsum(vals, idx, nseg):
    order = np.argsort(idx, kind='stable')
    sv = vals[order]
    si = idx[order]
    counts = np.bincount(si, minlength=nseg)
    out = np.zeros((nseg, vals.shape[1]), np.float32)
    nz = np.flatnonzero(counts)
    if nz.size:
        starts = np.concatenate([[0], np.cumsum(counts)])[nz]
        out[nz] = np.add.reduceat(sv, starts, axis=0)
    return out


def kernel(features, neighbour_distances, neighbour_list, triplet_idxs,
           angles, r_ij, r_ik, W_pre, W2b1, W2b2, W3b1, W3b2, W_post):
    d_all = np.asarray(neighbour_distances, np.float32)
    nl = np.asarray(neighbour_list)
    t1 = np.asarray(triplet_idxs)[:, 1]
    W2b1 = np.asarray(W2b1, np.float32)
    W2b2 = np.asarray(W2b2, np.float32)
    W3b1 = np.asarray(W3b1, np.float32)
    W3b2 = np.asarray(W3b2, np.float32)
    centers = np.linspace(0.0, CUTOFF, E, dtype=np.float32)

    # ---------------- edge host prep ----------------
    keep = d_all < CUTOFF
    kept_idx = np.flatnonzero(keep)
    d = d_all[kept_idx]
    b = d / DC
    cls = np.where(b < CB0, 0, np.where(b < CB1, 1, 2)).astype(np.int32)
    order = np.argsort(cls, kind='stable')
    kept_sorted = kept_idx[order]
    d_s = d[order]
    ncls = np.bincount(cls[order], minlength=3)

    nst_cls = [(int(n) + ST_E - 1) // ST_E for n in ncls]
    nst_tot = sum(nst_cls)
    nst_tot_pad = ((nst_tot + NCORES - 1) // NCORES) * NCORES
    nst_pc = nst_tot_pad // NCORES
    n_epad = nst_tot_pad * ST_E

    dP = np.zeros(n_epad, np.float32)
    zz = np.full(n_epad, -30.0, np.float32)
    st_cls = np.zeros(nst_tot_pad, np.int32)
    src_pos = np.full(n_epad, -1, np.int64)
    off_e = off_p = st_i = 0
    for c in range(3):
        n = int(ncls[c])
        shift = (W0S[c] + 15.5) * DC
        dseg = d_s[off_e:off_e + n] - shift
        env = 0.5 * (1.0 + np.cos(np.pi * d_s[off_e:off_e + n] / CUTOFF))
        dP[off_p:off_p + n] = dseg
        zz[off_p:off_p + n] = (np.log(np.maximum(env, 1e-35))
                               - GAMMA * dseg * dseg)
        src_pos[off_p:off_p + n] = off_e + np.arange(n)
        st_cls[st_i:st_i + nst_cls[c]] = c
        off_e += n
        off_p += nst_cls[c] * ST_E
        st_i += nst_cls[c]

    d_hi, d_lo = _hilo(dP)
    z_hi, z_lo = _hilo(zz)
    ones = np.ones(n_epad, BF16)
    zero = np.zeros(n_epad, BF16)
    # rows per chunk: [d_hi, d_lo, d_hi, z_hi, z_lo, 1, 1, 0]
    rows8 = np.stack([d_hi, d_lo, d_hi, z_hi, z_lo, ones, ones, zero])
    rows32 = (rows8.reshape(8, nst_tot_pad, 4, 512)
              .transpose(2, 0, 1, 3).reshape(32, nst_tot_pad * 512))

    aw_cls = np.zeros((3, 32, 128), BF16)
    w2_cls = np.zeros((3, 128, 128), BF16)
    for c in range(3):
        w0 = W0S[c]
        cp = centers[w0:w0 + 32] - (w0 + 15.5) * DC
        a_hi, a_lo = _hilo(2.0 * GAMMA * cp)
        b_hi, b_lo = _hilo(-GAMMA * cp * cp)
        one32 = np.ones(32, BF16)
        zero32 = np.zeros(32, BF16)
        blk = np.stack([a_hi, a_hi, a_lo, one32, one32, b_hi, b_lo, zero32])
        for j in range(4):
            aw_cls[c, 8 * j:8 * j + 8, 32 * j:32 * j + 32] = blk
        wwin = W2b1[w0:w0 + 32, :].astype(BF16)
        w2_cls[c, 0:32, 0:64] = wwin
        w2_cls[c, 32:64, 64:128] = wwin
        w2_cls[c, 64:96, 0:64] = wwin
        w2_cls[c, 96:128, 64:128] = wwin

    # ---------------- triplet host prep ----------------
    nmt_tot = (N_TRIP + MT_T - 1) // MT_T
    nmt_tot = ((nmt_tot + NCORES - 1) // NCORES) * NCORES
    ntp = nmt_tot * MT_T
    nmt_pc = nmt_tot // NCORES

    rij = np.zeros(ntp, np.float32)
    rik = np.zeros(ntp, np.float32)
    cosa = np.zeros(ntp, np.float32)
    rij[:N_TRIP] = np.asarray(r_ij, np.float32)
    rik[:N_TRIP] = np.asarray(r_ik, np.float32)
    cosa[:N_TRIP] = np.cos(np.asarray(angles, np.float32))
    rij_h, rij_l = _hilo(rij)
    rik_h, rik_l = _hilo(rik)
    cos_h, cos_l = _hilo(cosa)
    rows6 = np.stack([rij_h, rij_l, rik_h, rik_l, cos_h, cos_l])
    rows12 = (rows6.reshape(6, nmt_tot, 2, 512)
              .transpose(2, 0, 1, 3).reshape(12, nmt_tot * 512))

    w3rows = W3b1.astype(BF16)
    w3dup = np.stack([w3rows[0], w3rows[0], w3rows[1], w3rows[1],
                      w3rows[2], w3rows[2]])
    tw3_np = np.zeros((12, 128), BF16)
    tw3_np[0:6, 0:64] = w3dup
    tw3_np[6:12, 64:128] = w3dup

    # ---------------- build + run ----------------
    nc = _build(nst_pc, nmt_pc)
    in_maps = []
    for k in range(NCORES):
        es = slice(k * nst_pc * 512, (k + 1) * nst_pc * 512)
        sts = slice(k * nst_pc, (k + 1) * nst_pc)
        ts = slice(k * nmt_pc * 512, (k + 1) * nmt_pc * 512)
        ccls = st_cls[sts]
        in_maps.append({
            'erows': np.ascontiguousarray(rows32[:, es]),
            'eaw': np.ascontiguousarray(aw_cls[ccls]),
            'ew2': np.ascontiguousarray(w2_cls[ccls]),
            'trows': np.ascontiguousarray(rows12[:, ts]),
            'tw3': tw3_np,
        })
    res = bass_utils.run_bass_kernel_spmd(nc, in_maps, core_ids=list(range(NCORES)))
    kernel.last_results = res

    sT = np.concatenate([np.asarray(r['sT'], BF16) for r in res.results],
                        axis=1).astype(np.float32)
    uT = np.concatenate([np.asarray(r['uT'], BF16) for r in res.results],
                        axis=1).astype(np.float32)

    s_pad = (sT.reshape(2, 64, nst_tot_pad, 2, 512)
             .transpose(1, 2, 3, 0, 4).reshape(64, n_epad))
    u_pad = (uT.reshape(2, 64, nmt_tot, 512)
             .transpose(1, 2, 0, 3).reshape(64, ntp))

    # ---------------- host combine ----------------
    h = np.asarray(features, np.float32) @ np.asarray(W_pre, np.float32)

    valid = src_pos >= 0
    s_sorted = np.empty((int(ncls.sum()), 64), np.float32)
    s_sorted[src_pos[valid]] = s_pad[:, valid].T
    m_kept = s_sorted @ W2b2
    m_kept += (-LOG2) * W2b2.sum(axis=0)
    nl0_k = nl[0][kept_sorted]
    nl1_k = nl[1][kept_sorted]
    two_body = h[nl1_k] * m_kept
    agg = _segsum(two_body, nl0_k, N_NODES)

    u = u_pad[:, :N_TRIP].T
    U3 = _segsum(u, t1, N_NODES)
    U3 -= LOG2 * np.bincount(t1, minlength=N_NODES)[:, None].astype(np.float32)
    em = h[:N_NODES] * (U3 @ W3b2)
    agg += _segsum(em, nl[0][:N_NODES], N_NODES)

    return (agg @ np.asarray(W_post, np.float32)).astype(np.float32)


# revision 14
# speedup vs baseline: 1.3046x; 1.2789x over previous
"""M3GNet interaction kernel for 8 Trainium2 NeuronCores.

Device computes the dense per-edge radial-basis MLP activations and the
per-triplet angular MLP activations; edges and triplets are sharded 8 ways
(graph/data parallel). Host does the index-based gathers/segment sums and
the small channel-mixing matmuls.

Device-side structure (per core):
- Edge radial basis exploits Gaussian locality: each edge only sees a
  32-center window of the 64 RBF centers (3 overlapping window classes,
  edges bucketed by distance on host, class resolved via per-supertile
  stationary weights streamed as data). The exp argument
  -gamma*(d-c)^2 + ln(env) comes from one K=32 block-diagonal matmul per
  2048-edge supertile (4 chunks of 512 packed into 128 partitions), with
  hi/lo bf16 operand splitting for fp32-grade accuracy at full PE rate
  (the per-center -gamma*c'^2 bias is folded in as hi/lo constant rows).
- Triplet path: one K=12 block-diagonal matmul per 1024 triplets.
- softplus: native Softplus activation, or a staged Exp->Ln(1+x) pipeline
  batched by activation function to avoid ACT table-set thrash.
Outputs stream back as bf16.
"""
import numpy as np
import ml_dtypes

import concourse.bacc as bacc
import concourse.bass as bass
import concourse.mybir as mybir
from concourse.tile import TileContext
from concourse import bass_utils

BF16 = ml_dtypes.bfloat16

N_NODES = 20000
N_EDGES = 640000
N_TRIP = 1000000
C = 128
E = 64
CUTOFF = 5.0
LOG2 = float(np.log(2.0))
NCORES = 8
DC = CUTOFF / (E - 1)                       # center spacing
GAMMA = 1.0 / (2.0 * (CUTOFF / E) ** 2)
W0S = (0, 16, 32)                           # window starts per class
CB0, CB1 = 21.5, 37.5                       # class boundaries in bin units

ST_E = 2048       # edges per supertile (4 chunks of 512)
MT_T = 1024       # triplets per matmul tile (2 chunks of 512)

USE_SOFTPLUS = False     # no softplus ACT table in this toolchain

_CACHED = {}


def _hilo(x):
    x = np.asarray(x, np.float32)
    hi = x.astype(BF16)
    lo = (x - hi.astype(np.float32)).astype(BF16)
    return hi, lo


def _build(nst_pc, nmt_pc, use_softplus=USE_SOFTPLUS):
    key = (nst_pc, nmt_pc, use_softplus)
    if key in _CACHED:
        return _CACHED[key]
    nc = bacc.Bacc('TRN2', target_bir_lowering=False, debug=False)
    f32 = mybir.dt.float32
    bf16 = mybir.dt.bfloat16
    AF = mybir.ActivationFunctionType

    erows = nc.dram_tensor('erows', [32, nst_pc * 512], bf16, kind='ExternalInput')
    eaw = nc.dram_tensor('eaw', [nst_pc, 32, 128], bf16, kind='ExternalInput')
    ew2 = nc.dram_tensor('ew2', [nst_pc, 128, 128], bf16, kind='ExternalInput')
    trows = nc.dram_tensor('trows', [12, nmt_pc * 512], bf16, kind='ExternalInput')
    tw3 = nc.dram_tensor('tw3', [12, 128], bf16, kind='ExternalInput')

    sT = nc.dram_tensor('sT', [128, nst_pc * 1024], bf16, kind='ExternalOutput')
    uT = nc.dram_tensor('uT', [128, nmt_pc * 512], bf16, kind='ExternalOutput')

    n_eg = (nst_pc + 3) // 4      # edge stage-1 groups of 4 supertiles
    n_bt = (nst_pc + 1) // 2      # edge stage-2 psB tiles (2 supertiles each)
    n_ct = (nmt_pc + 3) // 4      # triplet psC tiles (4 mm tiles each)

    with TileContext(nc) as tc:
        with tc.tile_pool(name='rbe_w', bufs=1) as rbw:
            rbe = rbw.tile([128, nst_pc * 512], bf16, tag='rbe')

            # ---------- edge stage 1: arg matmuls + Exp -> rbe_wide ----------
            with (
                tc.tile_pool(name='e1_in', bufs=4) as e1i,
                tc.tile_pool(name='e1_w', bufs=4) as e1w,
                tc.tile_pool(name='e1_ps', bufs=2, space='PSUM') as ps1,
            ):
                for g in range(n_eg):
                    s0 = g * 4
                    ns = min(4, nst_pc - s0)
                    w = ns * 512
                    rows = e1i.tile([32, 2048], bf16, tag='erows')
                    nc.sync.dma_start(rows[:, 0:w],
                                      erows[:, s0 * 512:s0 * 512 + w])
                    awt = e1w.tile([32, 512], bf16, tag='aw')
                    nc.sync.dma_start(awt[:, 0:ns * 128],
                                      bass.AP(eaw, s0 * 32 * 128,
                                              [[128, 32], [4096, ns], [1, 128]]))
                    psA = ps1.tile([128, 2048], f32, tag='psA')
                    for j in range(ns):
                        nc.tensor.matmul(psA[:, j * 512:(j + 1) * 512],
                                         awt[:, j * 128:(j + 1) * 128],
                                         rows[:, j * 512:(j + 1) * 512])
                    nc.scalar.activation(rbe[:, s0 * 512:s0 * 512 + w],
                                         psA[:, 0:w], AF.Exp)

            # ---------- edge stage 2: p1 matmuls + softplus + out ----------
            with (
                tc.tile_pool(name='e2_w', bufs=3) as e2w,
                tc.tile_pool(name='e2_sb', bufs=2) as e2s,
                tc.tile_pool(name='e2_se', bufs=1) as e2e,
                tc.tile_pool(name='e2_ps', bufs=2, space='PSUM') as ps2,
            ):
                sexp = None
                if not use_softplus:
                    sexp = e2e.tile([128, nst_pc * 1024], bf16, tag='sexp')
                for b in range(n_bt):
                    s0 = b * 2
                    ns = min(2, nst_pc - s0)
                    w = ns * 1024
                    w2t = e2w.tile([128, 256], bf16, tag='w2')
                    nc.sync.dma_start(w2t[:, 0:ns * 128],
                                      bass.AP(ew2, s0 * 128 * 128,
                                              [[128, 128], [16384, ns], [1, 128]]))
                    psB = ps2.tile([128, 2048], f32, tag='psB')
                    for j in range(ns):
                        s = s0 + j
                        rb = rbe[:, s * 512:(s + 1) * 512]
                        nc.tensor.matmul(psB[:, j * 1024:j * 1024 + 512],
                                         w2t[0:64, j * 128:(j + 1) * 128],
                                         rb[0:64, :])
                        nc.tensor.matmul(psB[:, j * 1024 + 512:j * 1024 + 1024],
                                         w2t[64:128, j * 128:(j + 1) * 128],
                                         rb[64:128, :])
                    if use_softplus:
                        sout = e2s.tile([128, 2048], bf16, tag='sout')
                        nc.scalar.activation(sout[:, 0:w], psB[:, 0:w],
                                             AF.Softplus)
                        nc.sync.dma_start(sT[:, s0 * 1024:s0 * 1024 + w],
                                          sout[:, 0:w])
                    else:
                        nc.scalar.activation(sexp[:, s0 * 1024:s0 * 1024 + w],
                                             psB[:, 0:w], AF.Exp)
                if not use_softplus:
                    # gate: forces every Ln after the last Exp (Copy is in all
                    # ACT tables, so this adds no table switch)
                    gate = e2w.tile([128, 1], f32, tag='gate')
                    nc.scalar.activation(gate[:], sexp[:, nst_pc * 1024 - 1:nst_pc * 1024], AF.Copy,
                                         bias=1.0, scale=0.0)
                    ncols = nst_pc * 1024
                    step = 4096
                    for c0 in range(0, ncols, step):
                        w = min(step, ncols - c0)
                        sout = e2s.tile([128, step], bf16, tag='sout')
                        nc.scalar.activation(sout[:, 0:w], sexp[:, c0:c0 + w],
                                             AF.Ln, bias=1.0, scale=gate[:])
                        nc.sync.dma_start(sT[:, c0:c0 + w], sout[:, 0:w])

        # ---------- triplet phase ----------
        with (
            tc.tile_pool(name='t_w', bufs=1) as twp,
            tc.tile_pool(name='t_in', bufs=3) as tin,
            tc.tile_pool(name='t_sb', bufs=2) as tsb,
            tc.tile_pool(name='t_ue', bufs=1) as tue,
            tc.tile_pool(name='t_ps', bufs=2, space='PSUM') as tps,
        ):
            w3t = twp.tile([12, 128], bf16, tag='w3')
            nc.sync.dma_start(w3t[:], tw3[:])
            uexp = None
            if not use_softplus:
                uexp = tue.tile([128, nmt_pc * 512], bf16, tag='uexp')
            for b in range(n_ct):
                m0 = b * 4
                nm = min(4, nmt_pc - m0)
                w = nm * 512
                rows = tin.tile([12, 2048], bf16, tag='trows')
                nc.sync.dma_start(rows[:, 0:w],
                                  trows[:, m0 * 512:(m0 + nm) * 512])
                psC = tps.tile([128, 2048], f32, tag='psC')
                for j in range(nm):
                    nc.tensor.matmul(psC[:, j * 512:(j + 1) * 512],
                                     w3t[:], rows[:, j * 512:(j + 1) * 512])
                if use_softplus:
                    uout = tsb.tile([128, 2048], bf16, tag='uout')
                    nc.scalar.activation(uout[:, 0:w], psC[:, 0:w], AF.Softplus)
                    nc.sync.dma_start(uT[:, m0 * 512:(m0 + nm) * 512],
                                      uout[:, 0:w])
                else:
                    nc.scalar.activation(uexp[:, m0 * 512:m0 * 512 + w],
                                         psC[:, 0:w], AF.Exp)
            if not use_softplus:
                gate = twp.tile([128, 1], f32, tag='tgate')
                nc.scalar.activation(gate[:], uexp[:, nmt_pc * 512 - 1:nmt_pc * 512], AF.Copy,
                                     bias=1.0, scale=0.0)
                ncols = nmt_pc * 512
                step = 4096
                for c0 in range(0, ncols, step):
                    w = min(step, ncols - c0)
                    uout = tsb.tile([128, step], bf16, tag='uoutl')
                    nc.scalar.activation(uout[:, 0:w], uexp[:, c0:c0 + w],
                                         AF.Ln, bias=1.0, scale=gate[:])
                    nc.sync.dma_start(uT[:, c0:c0 + w], uout[:, 0:w])

    nc.compile()
    _CACHED[key] = nc
    return nc


def _segsum(vals, idx, nseg):
    order = np.argsort(idx, kind='stable')
    sv = vals[order]
    si = idx[order]
    counts = np.bincount(si, minlength=nseg)
    out = np.zeros((nseg, vals.shape[1]), np.float32)
    nz = np.flatnonzero(counts)
    if nz.size:
        starts = np.concatenate([[0], np.cumsum(counts)])[nz]
        out[nz] = np.add.reduceat(sv, starts, axis=0)
    return out


def kernel(features, neighbour_distances, neighbour_list, triplet_idxs,
           angles, r_ij, r_ik, W_pre, W2b1, W2b2, W3b1, W3b2, W_post):
    d_all = np.asarray(neighbour_distances, np.float32)
    nl = np.asarray(neighbour_list)
    t1 = np.asarray(triplet_idxs)[:, 1]
    W2b1 = np.asarray(W2b1, np.float32)
    W2b2 = np.asarray(W2b2, np.float32)
    W3b1 = np.asarray(W3b1, np.float32)
    W3b2 = np.asarray(W3b2, np.float32)
    centers = np.linspace(0.0, CUTOFF, E, dtype=np.float32)

    # ---------------- edge host prep ----------------
    keep = d_all < CUTOFF
    kept_idx = np.flatnonzero(keep)
    d = d_all[kept_idx]
    b = d / DC
    cls = np.where(b < CB0, 0, np.where(b < CB1, 1, 2)).astype(np.int32)
    order = np.argsort(cls, kind='stable')
    kept_sorted = kept_idx[order]
    d_s = d[order]
    ncls = np.bincount(cls[order], minlength=3)

    nst_cls = [(int(n) + ST_E - 1) // ST_E for n in ncls]
    nst_tot = sum(nst_cls)
    nst_tot_pad = ((nst_tot + NCORES - 1) // NCORES) * NCORES
    nst_pc = nst_tot_pad // NCORES
    n_epad = nst_tot_pad * ST_E

    dP = np.zeros(n_epad, np.float32)
    zz = np.full(n_epad, -30.0, np.float32)
    st_cls = np.zeros(nst_tot_pad, np.int32)
    src_pos = np.full(n_epad, -1, np.int64)
    off_e = off_p = st_i = 0
    for c in range(3):
        n = int(ncls[c])
        shift = (W0S[c] + 15.5) * DC
        dseg = d_s[off_e:off_e + n] - shift
        env = 0.5 * (1.0 + np.cos(np.pi * d_s[off_e:off_e + n] / CUTOFF))
        dP[off_p:off_p + n] = dseg
        zz[off_p:off_p + n] = (np.log(np.maximum(env, 1e-35))
                               - GAMMA * dseg * dseg)
        src_pos[off_p:off_p + n] = off_e + np.arange(n)
        st_cls[st_i:st_i + nst_cls[c]] = c
        off_e += n
        off_p += nst_cls[c] * ST_E
        st_i += nst_cls[c]

    d_hi, d_lo = _hilo(dP)
    z_hi, z_lo = _hilo(zz)
    ones = np.ones(n_epad, BF16)
    zero = np.zeros(n_epad, BF16)
    # rows per chunk: [d_hi, d_lo, d_hi, z_hi, z_lo, 1, 1, 0]
    rows8 = np.stack([d_hi, d_lo, d_hi, z_hi, z_lo, ones, ones, zero])
    rows32 = (rows8.reshape(8, nst_tot_pad, 4, 512)
              .transpose(2, 0, 1, 3).reshape(32, nst_tot_pad * 512))

    aw_cls = np.zeros((3, 32, 128), BF16)
    w2_cls = np.zeros((3, 128, 128), BF16)
    for c in range(3):
        w0 = W0S[c]
        cp = centers[w0:w0 + 32] - (w0 + 15.5) * DC
        a_hi, a_lo = _hilo(2.0 * GAMMA * cp)
        b_hi, b_lo = _hilo(-GAMMA * cp * cp)
        one32 = np.ones(32, BF16)
        zero32 = np.zeros(32, BF16)
        blk = np.stack([a_hi, a_hi, a_lo, one32, one32, b_hi, b_lo, zero32])
        for j in range(4):
            aw_cls[c, 8 * j:8 * j + 8, 32 * j:32 * j + 32] = blk
        wwin = W2b1[w0:w0 + 32, :].astype(BF16)
        w2_cls[c, 0:32, 0:64] = wwin
        w2_cls[c, 32:64, 64:128] = wwin
        w2_cls[c, 64:96, 0:64] = wwin
        w2_cls[c, 96:128, 64:128] = wwin

    # ---------------- triplet host prep ----------------
    nmt_tot = (N_TRIP + MT_T - 1) // MT_T
    nmt_tot = ((nmt_tot + NCORES - 1) // NCORES) * NCORES
    ntp = nmt_tot * MT_T
    nmt_pc = nmt_tot // NCORES

    rij = np.zeros(ntp, np.float32)
    rik = np.zeros(ntp, np.float32)
    cosa = np.zeros(ntp, np.float32)
    rij[:N_TRIP] = np.asarray(r_ij, np.float32)
    rik[:N_TRIP] = np.asarray(r_ik, np.float32)
    cosa[:N_TRIP] = np.cos(np.asarray(angles, np.float32))
    rij_h, rij_l = _hilo(rij)
    rik_h, rik_l = _hilo(rik)
    cos_h, cos_l = _hilo(cosa)
    rows6 = np.stack([rij_h, rij_l, rik_h, rik_l, cos_h, cos_l])
    rows12 = (rows6.reshape(6, nmt_tot, 2, 512)
              .transpose(2, 0, 1, 3).reshape(12, nmt_tot * 512))

    w3rows = W3b1.astype(BF16)
    w3dup = np.stack([w3rows[0], w3rows[0], w3rows[1], w3rows[1],
                      w3rows[2], w3rows[2]])
    tw3_np = np.zeros((12, 128), BF16)
    tw3_np[0:6, 0:64] = w3dup
    tw3_np[6:12, 64:128] = w3dup

    # ---------------- build + run ----------------
    nc = _build(nst_pc, nmt_pc)
    in_maps = []
    for k in range(NCORES):
        es = slice(k * nst_pc * 512, (k + 1) * nst_pc * 512)
        sts = slice(k * nst_pc, (k + 1) * nst_pc)
        ts = slice(k * nmt_pc * 512, (k + 1) * nmt_pc * 512)
        ccls = st_cls[sts]
        in_maps.append({
            'erows': np.ascontiguousarray(rows32[:, es]),
            'eaw': np.ascontiguousarray(aw_cls[ccls]),
            'ew2': np.ascontiguousarray(w2_cls[ccls]),
            'trows': np.ascontiguousarray(rows12[:, ts]),
            'tw3': tw3_np,
        })
    res = bass_utils.run_bass_kernel_spmd(nc, in_maps, core_ids=list(range(NCORES)))
    kernel.last_results = res

    sT = np.concatenate([np.asarray(r['sT'], BF16) for r in res.results],
                        axis=1).astype(np.float32)
    uT = np.concatenate([np.asarray(r['uT'], BF16) for r in res.results],
                        axis=1).astype(np.float32)

    s_pad = (sT.reshape(2, 64, nst_tot_pad, 2, 512)
             .transpose(1, 2, 3, 0, 4).reshape(64, n_epad))
    u_pad = (uT.reshape(2, 64, nmt_tot, 512)
             .transpose(1, 2, 0, 3).reshape(64, ntp))

    # ---------------- host combine ----------------
    h = np.asarray(features, np.float32) @ np.asarray(W_pre, np.float32)

    valid = src_pos >= 0
    s_sorted = np.empty((int(ncls.sum()), 64), np.float32)
    s_sorted[src_pos[valid]] = s_pad[:, valid].T
    m_kept = s_sorted @ W2b2
    m_kept += (-LOG2) * W2b2.sum(axis=0)
    nl0_k = nl[0][kept_sorted]
    nl1_k = nl[1][kept_sorted]
    two_body = h[nl1_k] * m_kept
    agg = _segsum(two_body, nl0_k, N_NODES)

    u = u_pad[:, :N_TRIP].T
    U3 = _segsum(u, t1, N_NODES)
    U3 -= LOG2 * np.bincount(t1, minlength=N_NODES)[:, None].astype(np.float32)
    em = h[:N_NODES] * (U3 @ W3b2)
    agg += _segsum(em, nl[0][:N_NODES], N_NODES)

    return (agg @ np.asarray(W_post, np.float32)).astype(np.float32)


# revision 15
# speedup vs baseline: 1.3187x; 1.0108x over previous
"""M3GNet interaction kernel for 8 Trainium2 NeuronCores.

Device computes the dense per-edge radial-basis MLP activations and the
per-triplet angular MLP activations; edges and triplets are sharded 8 ways
(graph/data parallel). Host does the index-based gathers/segment sums and
the small channel-mixing matmuls.

Device-side structure (per core):
- Edge radial basis exploits Gaussian locality: each edge only sees a
  32-center window of the 64 RBF centers (3 overlapping window classes,
  edges bucketed by distance on host, class resolved via per-supertile
  stationary weights streamed as data). The exp argument
  -gamma*(d-c)^2 + ln(env) comes from one K=32 block-diagonal matmul per
  2048-edge supertile (4 chunks of 512 packed into 128 partitions), with
  hi/lo bf16 operand splitting for fp32-grade accuracy at full PE rate
  (the per-center -gamma*c'^2 bias is folded in as hi/lo constant rows).
- Triplet path: one K=12 block-diagonal matmul per 1024 triplets.
- softplus: native Softplus activation, or a staged Exp->Ln(1+x) pipeline
  batched by activation function to avoid ACT table-set thrash.
Outputs stream back as bf16.
"""
import numpy as np
import ml_dtypes

import concourse.bacc as bacc
import concourse.bass as bass
import concourse.mybir as mybir
from concourse.tile import TileContext
from concourse import bass_utils

BF16 = ml_dtypes.bfloat16

N_NODES = 20000
N_EDGES = 640000
N_TRIP = 1000000
C = 128
E = 64
CUTOFF = 5.0
LOG2 = float(np.log(2.0))
NCORES = 8
DC = CUTOFF / (E - 1)                       # center spacing
GAMMA = 1.0 / (2.0 * (CUTOFF / E) ** 2)
W0S = (0, 16, 32)                           # window starts per class
CB0, CB1 = 21.5, 37.5                       # class boundaries in bin units

ST_E = 2048       # edges per supertile (4 chunks of 512)
MT_T = 1024       # triplets per matmul tile (2 chunks of 512)

USE_SOFTPLUS = False     # no softplus ACT table in this toolchain

_CACHED = {}


def _hilo(x):
    x = np.asarray(x, np.float32)
    hi = x.astype(BF16)
    lo = (x - hi.astype(np.float32)).astype(BF16)
    return hi, lo


def _build(nst_pc, nmt_pc, use_softplus=USE_SOFTPLUS):
    key = (nst_pc, nmt_pc, use_softplus)
    if key in _CACHED:
        return _CACHED[key]
    nc = bacc.Bacc('TRN2', target_bir_lowering=False, debug=False)
    f32 = mybir.dt.float32
    bf16 = mybir.dt.bfloat16
    AF = mybir.ActivationFunctionType

    erows = nc.dram_tensor('erows', [32, nst_pc * 512], bf16, kind='ExternalInput')
    eaw = nc.dram_tensor('eaw', [nst_pc, 32, 128], bf16, kind='ExternalInput')
    ew2 = nc.dram_tensor('ew2', [nst_pc, 128, 128], bf16, kind='ExternalInput')
    trows = nc.dram_tensor('trows', [12, nmt_pc * 512], bf16, kind='ExternalInput')
    tw3 = nc.dram_tensor('tw3', [12, 128], bf16, kind='ExternalInput')

    sT = nc.dram_tensor('sT', [128, nst_pc * 1024], bf16, kind='ExternalOutput')
    uT = nc.dram_tensor('uT', [128, nmt_pc * 512], bf16, kind='ExternalOutput')

    n_eg = (nst_pc + 3) // 4      # edge stage-1 groups of 4 supertiles
    n_bt = (nst_pc + 1) // 2      # edge stage-2 psB tiles (2 supertiles each)
    n_ct = (nmt_pc + 3) // 4      # triplet psC tiles (4 mm tiles each)

    with TileContext(nc) as tc:
        with tc.tile_pool(name='rbe_w', bufs=1) as rbw:
            warm = rbw.tile([128, 8], f32, tag='warm')
            nc.vector.tensor_scalar(warm[:], warm[:], 0.0, None,
                                    mybir.AluOpType.mult)
            nc.scalar.activation(warm[:], warm[:], AF.Exp)
            rbe = rbw.tile([128, nst_pc * 512], bf16, tag='rbe')

            # ---------- edge stage 1: arg matmuls + Exp -> rbe_wide ----------
            with (
                tc.tile_pool(name='e1_in', bufs=4) as e1i,
                tc.tile_pool(name='e1_w', bufs=4) as e1w,
                tc.tile_pool(name='e1_ps', bufs=2, space='PSUM') as ps1,
            ):
                for g in range(n_eg):
                    s0 = g * 4
                    ns = min(4, nst_pc - s0)
                    w = ns * 512
                    rows = e1i.tile([32, 2048], bf16, tag='erows')
                    nc.sync.dma_start(rows[:, 0:w],
                                      erows[:, s0 * 512:s0 * 512 + w])
                    awt = e1w.tile([32, 512], bf16, tag='aw')
                    nc.sync.dma_start(awt[:, 0:ns * 128],
                                      bass.AP(eaw, s0 * 32 * 128,
                                              [[128, 32], [4096, ns], [1, 128]]))
                    psA = ps1.tile([128, 2048], f32, tag='psA')
                    for j in range(ns):
                        nc.tensor.matmul(psA[:, j * 512:(j + 1) * 512],
                                         awt[:, j * 128:(j + 1) * 128],
                                         rows[:, j * 512:(j + 1) * 512])
                    nc.scalar.activation(rbe[:, s0 * 512:s0 * 512 + w],
                                         psA[:, 0:w], AF.Exp)

            # ---------- edge stage 2: p1 matmuls + softplus + out ----------
            with (
                tc.tile_pool(name='e2_w', bufs=3) as e2w,
                tc.tile_pool(name='e2_sb', bufs=2) as e2s,
                tc.tile_pool(name='e2_se', bufs=1) as e2e,
                tc.tile_pool(name='e2_ps', bufs=2, space='PSUM') as ps2,
            ):
                sexp = None
                if not use_softplus:
                    sexp = e2e.tile([128, nst_pc * 1024], bf16, tag='sexp')
                for b in range(n_bt):
                    s0 = b * 2
                    ns = min(2, nst_pc - s0)
                    w = ns * 1024
                    w2t = e2w.tile([128, 256], bf16, tag='w2')
                    nc.sync.dma_start(w2t[:, 0:ns * 128],
                                      bass.AP(ew2, s0 * 128 * 128,
                                              [[128, 128], [16384, ns], [1, 128]]))
                    psB = ps2.tile([128, 2048], f32, tag='psB')
                    for j in range(ns):
                        s = s0 + j
                        rb = rbe[:, s * 512:(s + 1) * 512]
                        nc.tensor.matmul(psB[:, j * 1024:j * 1024 + 512],
                                         w2t[0:64, j * 128:(j + 1) * 128],
                                         rb[0:64, :])
                        nc.tensor.matmul(psB[:, j * 1024 + 512:j * 1024 + 1024],
                                         w2t[64:128, j * 128:(j + 1) * 128],
                                         rb[64:128, :])
                    if use_softplus:
                        sout = e2s.tile([128, 2048], bf16, tag='sout')
                        nc.scalar.activation(sout[:, 0:w], psB[:, 0:w],
                                             AF.Softplus)
                        nc.sync.dma_start(sT[:, s0 * 1024:s0 * 1024 + w],
                                          sout[:, 0:w])
                    else:
                        nc.scalar.activation(sexp[:, s0 * 1024:s0 * 1024 + w],
                                             psB[:, 0:w], AF.Exp)
                if not use_softplus:
                    # gate: forces every Ln after the last Exp (Copy is in all
                    # ACT tables, so this adds no table switch)
                    gate = e2w.tile([128, 1], f32, tag='gate')
                    nc.scalar.activation(gate[:], sexp[:, nst_pc * 1024 - 1:nst_pc * 1024], AF.Copy,
                                         bias=1.0, scale=0.0)
                    ncols = nst_pc * 1024
                    step = 8192
                    for c0 in range(0, ncols, step):
                        w = min(step, ncols - c0)
                        sout = e2s.tile([128, step], bf16, tag='sout')
                        nc.scalar.activation(sout[:, 0:w], sexp[:, c0:c0 + w],
                                             AF.Ln, bias=1.0, scale=gate[:])
                        nc.sync.dma_start(sT[:, c0:c0 + w], sout[:, 0:w])

        # ---------- triplet phase ----------
        with (
            tc.tile_pool(name='t_w', bufs=1) as twp,
            tc.tile_pool(name='t_in', bufs=4) as tin,
            tc.tile_pool(name='t_sb', bufs=2) as tsb,
            tc.tile_pool(name='t_ue', bufs=1) as tue,
            tc.tile_pool(name='t_ps', bufs=2, space='PSUM') as tps,
        ):
            w3t = twp.tile([12, 128], bf16, tag='w3')
            nc.sync.dma_start(w3t[:], tw3[:])
            uexp = None
            if not use_softplus:
                uexp = tue.tile([128, nmt_pc * 512], bf16, tag='uexp')
            for b in range(n_ct):
                m0 = b * 4
                nm = min(4, nmt_pc - m0)
                w = nm * 512
                rows = tin.tile([12, 2048], bf16, tag='trows')
                nc.sync.dma_start(rows[:, 0:w],
                                  trows[:, m0 * 512:(m0 + nm) * 512])
                psC = tps.tile([128, 2048], f32, tag='psC')
                for j in range(nm):
                    nc.tensor.matmul(psC[:, j * 512:(j + 1) * 512],
                                     w3t[:], rows[:, j * 512:(j + 1) * 512])
                if use_softplus:
                    uout = tsb.tile([128, 2048], bf16, tag='uout')
                    nc.scalar.activation(uout[:, 0:w], psC[:, 0:w], AF.Softplus)
                    nc.sync.dma_start(uT[:, m0 * 512:(m0 + nm) * 512],
                                      uout[:, 0:w])
                else:
                    nc.scalar.activation(uexp[:, m0 * 512:m0 * 512 + w],
                                         psC[:, 0:w], AF.Exp)
            if not use_softplus:
                gate = twp.tile([128, 1], f32, tag='tgate')
                nc.scalar.activation(gate[:], uexp[:, nmt_pc * 512 - 1:nmt_pc * 512], AF.Copy,
                                     bias=1.0, scale=0.0)
                ncols = nmt_pc * 512
                step = 8192
                for c0 in range(0, ncols, step):
                    w = min(step, ncols - c0)
                    uout = tsb.tile([128, step], bf16, tag='uoutl')
                    nc.scalar.activation(uout[:, 0:w], uexp[:, c0:c0 + w],
                                         AF.Ln, bias=1.0, scale=gate[:])
                    nc.sync.dma_start(uT[:, c0:c0 + w], uout[:, 0:w])

    nc.compile()
    _CACHED[key] = nc
    return nc


def _segsum(vals, idx, nseg):
    order = np.argsort(idx, kind='stable')
    sv = vals[order]
    si = idx[order]
    counts = np.bincount(si, minlength=nseg)
    out = np.zeros((nseg, vals.shape[1]), np.float32)
    nz = np.flatnonzero(counts)
    if nz.size:
        starts = np.concatenate([[0], np.cumsum(counts)])[nz]
        out[nz] = np.add.reduceat(sv, starts, axis=0)
    return out


def kernel(features, neighbour_distances, neighbour_list, triplet_idxs,
           angles, r_ij, r_ik, W_pre, W2b1, W2b2, W3b1, W3b2, W_post):
    d_all = np.asarray(neighbour_distances, np.float32)
    nl = np.asarray(neighbour_list)
    t1 = np.asarray(triplet_idxs)[:, 1]
    W2b1 = np.asarray(W2b1, np.float32)
    W2b2 = np.asarray(W2b2, np.float32)
    W3b1 = np.asarray(W3b1, np.float32)
    W3b2 = np.asarray(W3b2, np.float32)
    centers = np.linspace(0.0, CUTOFF, E, dtype=np.float32)

    # ---------------- edge host prep ----------------
    keep = d_all < CUTOFF
    kept_idx = np.flatnonzero(keep)
    d = d_all[kept_idx]
    b = d / DC
    cls = np.where(b < CB0, 0, np.where(b < CB1, 1, 2)).astype(np.int32)
    order = np.argsort(cls, kind='stable')
    kept_sorted = kept_idx[order]
    d_s = d[order]
    ncls = np.bincount(cls[order], minlength=3)

    nst_cls = [(int(n) + ST_E - 1) // ST_E for n in ncls]
    nst_tot = sum(nst_cls)
    nst_tot_pad = ((nst_tot + NCORES - 1) // NCORES) * NCORES
    nst_pc = nst_tot_pad // NCORES
    n_epad = nst_tot_pad * ST_E

    dP = np.zeros(n_epad, np.float32)
    zz = np.full(n_epad, -30.0, np.float32)
    st_cls = np.zeros(nst_tot_pad, np.int32)
    src_pos = np.full(n_epad, -1, np.int64)
    off_e = off_p = st_i = 0
    for c in range(3):
        n = int(ncls[c])
        shift = (W0S[c] + 15.5) * DC
        dseg = d_s[off_e:off_e + n] - shift
        env = 0.5 * (1.0 + np.cos(np.pi * d_s[off_e:off_e + n] / CUTOFF))
        dP[off_p:off_p + n] = dseg
        zz[off_p:off_p + n] = (np.log(np.maximum(env, 1e-35))
                               - GAMMA * dseg * dseg)
        src_pos[off_p:off_p + n] = off_e + np.arange(n)
        st_cls[st_i:st_i + nst_cls[c]] = c
        off_e += n
        off_p += nst_cls[c] * ST_E
        st_i += nst_cls[c]

    d_hi, d_lo = _hilo(dP)
    z_hi, z_lo = _hilo(zz)
    ones = np.ones(n_epad, BF16)
    zero = np.zeros(n_epad, BF16)
    # rows per chunk: [d_hi, d_lo, d_hi, z_hi, z_lo, 1, 1, 0]
    rows8 = np.stack([d_hi, d_lo, d_hi, z_hi, z_lo, ones, ones, zero])
    rows32 = (rows8.reshape(8, nst_tot_pad, 4, 512)
              .transpose(2, 0, 1, 3).reshape(32, nst_tot_pad * 512))

    aw_cls = np.zeros((3, 32, 128), BF16)
    w2_cls = np.zeros((3, 128, 128), BF16)
    for c in range(3):
        w0 = W0S[c]
        cp = centers[w0:w0 + 32] - (w0 + 15.5) * DC
        a_hi, a_lo = _hilo(2.0 * GAMMA * cp)
        b_hi, b_lo = _hilo(-GAMMA * cp * cp)
        one32 = np.ones(32, BF16)
        zero32 = np.zeros(32, BF16)
        blk = np.stack([a_hi, a_hi, a_lo, one32, one32, b_hi, b_lo, zero32])
        for j in range(4):
            aw_cls[c, 8 * j:8 * j + 8, 32 * j:32 * j + 32] = blk
        wwin = W2b1[w0:w0 + 32, :].astype(BF16)
        w2_cls[c, 0:32, 0:64] = wwin
        w2_cls[c, 32:64, 64:128] = wwin
        w2_cls[c, 64:96, 0:64] = wwin
        w2_cls[c, 96:128, 64:128] = wwin

    # ---------------- triplet host prep ----------------
    nmt_tot = (N_TRIP + MT_T - 1) // MT_T
    nmt_tot = ((nmt_tot + NCORES - 1) // NCORES) * NCORES
    ntp = nmt_tot * MT_T
    nmt_pc = nmt_tot // NCORES

    rij = np.zeros(ntp, np.float32)
    rik = np.zeros(ntp, np.float32)
    cosa = np.zeros(ntp, np.float32)
    rij[:N_TRIP] = np.asarray(r_ij, np.float32)
    rik[:N_TRIP] = np.asarray(r_ik, np.float32)
    cosa[:N_TRIP] = np.cos(np.asarray(angles, np.float32))
    rij_h, rij_l = _hilo(rij)
    rik_h, rik_l = _hilo(rik)
    cos_h, cos_l = _hilo(cosa)
    rows6 = np.stack([rij_h, rij_l, rik_h, rik_l, cos_h, cos_l])
    rows12 = (rows6.reshape(6, nmt_tot, 2, 512)
              .transpose(2, 0, 1, 3).reshape(12, nmt_tot * 512))

    w3rows = W3b1.astype(BF16)
    w3dup = np.stack([w3rows[0], w3rows[0], w3rows[1], w3rows[1],
                      w3rows[2], w3rows[2]])
    tw3_np = np.zeros((12, 128), BF16)
    tw3_np[0:6, 0:64] = w3dup
    tw3_np[6:12, 64:128] = w3dup

    # ---------------- build + run ----------------
    nc = _build(nst_pc, nmt_pc)
    in_maps = []
    for k in range(NCORES):
        es = slice(k * nst_pc * 512, (k + 1) * nst_pc * 512)
        sts = slice(k * nst_pc, (k + 1) * nst_pc)
        ts = slice(k * nmt_pc * 512, (k + 1) * nmt_pc * 512)
        ccls = st_cls[sts]
        in_maps.append({
            'erows': np.ascontiguousarray(rows32[:, es]),
            'eaw': np.ascontiguousarray(aw_cls[ccls]),
            'ew2': np.ascontiguousarray(w2_cls[ccls]),
            'trows': np.ascontiguousarray(rows12[:, ts]),
            'tw3': tw3_np,
        })
    res = bass_utils.run_bass_kernel_spmd(nc, in_maps, core_ids=list(range(NCORES)))
    kernel.last_results = res

    sT = np.concatenate([np.asarray(r['sT'], BF16) for r in res.results],
                        axis=1).astype(np.float32)
    uT = np.concatenate([np.asarray(r['uT'], BF16) for r in res.results],
                        axis=1).astype(np.float32)

    s_pad = (sT.reshape(2, 64, nst_tot_pad, 2, 512)
             .transpose(1, 2, 3, 0, 4).reshape(64, n_epad))
    u_pad = (uT.reshape(2, 64, nmt_tot, 512)
             .transpose(1, 2, 0, 3).reshape(64, ntp))

    # ---------------- host combine ----------------
    h = np.asarray(features, np.float32) @ np.asarray(W_pre, np.float32)

    valid = src_pos >= 0
    s_sorted = np.empty((int(ncls.sum()), 64), np.float32)
    s_sorted[src_pos[valid]] = s_pad[:, valid].T
    m_kept = s_sorted @ W2b2
    m_kept += (-LOG2) * W2b2.sum(axis=0)
    nl0_k = nl[0][kept_sorted]
    nl1_k = nl[1][kept_sorted]
    two_body = h[nl1_k] * m_kept
    agg = _segsum(two_body, nl0_k, N_NODES)

    u = u_pad[:, :N_TRIP].T
    U3 = _segsum(u, t1, N_NODES)
    U3 -= LOG2 * np.bincount(t1, minlength=N_NODES)[:, None].astype(np.float32)
    em = h[:N_NODES] * (U3 @ W3b2)
    agg += _segsum(em, nl[0][:N_NODES], N_NODES)

    return (agg @ np.asarray(W_post, np.float32)).astype(np.float32)


# revision 16
# speedup vs baseline: 1.3561x; 1.0283x over previous
"""M3GNet interaction kernel for 8 Trainium2 NeuronCores.

Device computes the dense per-edge radial-basis MLP activations and the
per-triplet angular MLP activations; edges and triplets are sharded 8 ways
(graph/data parallel). Host does the index-based gathers/segment sums and
the small channel-mixing matmuls.

Device-side structure (per core):
- Edge radial basis exploits Gaussian locality: each edge only sees a
  32-center window of the 64 RBF centers (3 overlapping window classes,
  edges bucketed by distance on host, class resolved via per-supertile
  stationary weights streamed as data). The exp argument
  -gamma*(d-c)^2 + ln(env) comes from one K=32 block-diagonal matmul per
  2048-edge supertile (4 chunks of 512 packed into 128 partitions), with
  hi/lo bf16 operand splitting for fp32-grade accuracy at full PE rate
  (the per-center -gamma*c'^2 bias is folded in as hi/lo constant rows).
- Triplet path: one K=12 block-diagonal matmul per 1024 triplets.
- softplus: native Softplus activation, or a staged Exp->Ln(1+x) pipeline
  batched by activation function to avoid ACT table-set thrash.
Outputs stream back as bf16.
"""
import sys
import types

import numpy as np
import ml_dtypes

import concourse.bacc as bacc
import concourse.bass as bass
import concourse.mybir as mybir
from concourse.tile import TileContext
from concourse import bass_utils

try:  # pragma: no cover - environment shim
    import antenv.axon_hooks  # noqa: F401
except ImportError:
    # bass_utils imports antenv.axon_hooks unconditionally when BASS_TRACE=1
    # under axon; provide a no-op hook module so tracing degrades gracefully
    # instead of crashing in environments without it.
    try:
        import antenv
        _mod = types.ModuleType('antenv.axon_hooks')
        _mod._hook = None
        _mod.set_axon_ntff_profile_hook = lambda h: setattr(_mod, '_hook', h)
        _mod.get_axon_ntff_profile_hook = lambda: _mod._hook
        sys.modules['antenv.axon_hooks'] = _mod
        antenv.axon_hooks = _mod
    except Exception:
        pass

BF16 = ml_dtypes.bfloat16

N_NODES = 20000
N_EDGES = 640000
N_TRIP = 1000000
C = 128
E = 64
CUTOFF = 5.0
LOG2 = float(np.log(2.0))
NCORES = 8
DC = CUTOFF / (E - 1)                       # center spacing
GAMMA = 1.0 / (2.0 * (CUTOFF / E) ** 2)
W0S = (0, 16, 32)                           # window starts per class
CB0, CB1 = 21.5, 37.5                       # class boundaries in bin units

ST_E = 2048       # edges per supertile (4 chunks of 512)
MT_T = 1024       # triplets per matmul tile (2 chunks of 512)

USE_SOFTPLUS = False     # no softplus ACT table in this toolchain

_CACHED = {}


def _hilo(x):
    x = np.asarray(x, np.float32)
    hi = x.astype(BF16)
    lo = (x - hi.astype(np.float32)).astype(BF16)
    return hi, lo


def _build(nst_pc, nmt_pc, use_softplus=USE_SOFTPLUS):
    key = (nst_pc, nmt_pc, use_softplus)
    if key in _CACHED:
        return _CACHED[key]
    nc = bacc.Bacc('TRN2', target_bir_lowering=False, debug=False)
    f32 = mybir.dt.float32
    bf16 = mybir.dt.bfloat16
    AF = mybir.ActivationFunctionType

    erows = nc.dram_tensor('erows', [32, nst_pc * 512], bf16, kind='ExternalInput')
    eaw = nc.dram_tensor('eaw', [nst_pc, 32, 128], bf16, kind='ExternalInput')
    ew2 = nc.dram_tensor('ew2', [nst_pc, 128, 128], bf16, kind='ExternalInput')
    trows = nc.dram_tensor('trows', [12, nmt_pc * 512], bf16, kind='ExternalInput')
    tw3 = nc.dram_tensor('tw3', [12, 128], bf16, kind='ExternalInput')

    sT = nc.dram_tensor('sT', [128, nst_pc * 1024], bf16, kind='ExternalOutput')
    uT = nc.dram_tensor('uT', [128, nmt_pc * 512], bf16, kind='ExternalOutput')

    n_eg = (nst_pc + 3) // 4      # edge stage-1 groups of 4 supertiles
    n_bt = (nst_pc + 1) // 2      # edge stage-2 psB tiles (2 supertiles each)
    n_ct = (nmt_pc + 3) // 4      # triplet psC tiles (4 mm tiles each)

    with TileContext(nc) as tc:
        with tc.tile_pool(name='rbe_w', bufs=1) as rbw:
            warm = rbw.tile([128, 8], f32, tag='warm')
            nc.vector.tensor_scalar(warm[:], warm[:], 0.0, None,
                                    mybir.AluOpType.mult)
            nc.scalar.activation(warm[:], warm[:], AF.Exp)
            rbe = rbw.tile([128, nst_pc * 512], bf16, tag='rbe')

            # ---------- edge stage 1: arg matmuls + Exp -> rbe_wide ----------
            with (
                tc.tile_pool(name='e1_in', bufs=4) as e1i,
                tc.tile_pool(name='e1_w', bufs=4) as e1w,
                tc.tile_pool(name='e1_ps', bufs=2, space='PSUM') as ps1,
            ):
                for g in range(n_eg):
                    s0 = g * 4
                    ns = min(4, nst_pc - s0)
                    w = ns * 512
                    rows = e1i.tile([32, 2048], bf16, tag='erows')
                    nc.sync.dma_start(rows[:, 0:w],
                                      erows[:, s0 * 512:s0 * 512 + w])
                    awt = e1w.tile([32, 512], bf16, tag='aw')
                    nc.sync.dma_start(awt[:, 0:ns * 128],
                                      bass.AP(eaw, s0 * 32 * 128,
                                              [[128, 32], [4096, ns], [1, 128]]))
                    psA = ps1.tile([128, 2048], f32, tag='psA')
                    for j in range(ns):
                        nc.tensor.matmul(psA[:, j * 512:(j + 1) * 512],
                                         awt[:, j * 128:(j + 1) * 128],
                                         rows[:, j * 512:(j + 1) * 512])
                    nc.scalar.activation(rbe[:, s0 * 512:s0 * 512 + w],
                                         psA[:, 0:w], AF.Exp)

            # ---------- edge stage 2: p1 matmuls + softplus + out ----------
            with (
                tc.tile_pool(name='e2_w', bufs=3) as e2w,
                tc.tile_pool(name='e2_sb', bufs=2) as e2s,
                tc.tile_pool(name='e2_se', bufs=1) as e2e,
                tc.tile_pool(name='e2_ps', bufs=2, space='PSUM') as ps2,
            ):
                sexp = None
                if not use_softplus:
                    sexp = e2e.tile([128, nst_pc * 1024], bf16, tag='sexp')
                for b in range(n_bt):
                    s0 = b * 2
                    ns = min(2, nst_pc - s0)
                    w = ns * 1024
                    w2t = e2w.tile([128, 256], bf16, tag='w2')
                    nc.sync.dma_start(w2t[:, 0:ns * 128],
                                      bass.AP(ew2, s0 * 128 * 128,
                                              [[128, 128], [16384, ns], [1, 128]]))
                    psB = ps2.tile([128, 2048], f32, tag='psB')
                    for j in range(ns):
                        s = s0 + j
                        rb = rbe[:, s * 512:(s + 1) * 512]
                        nc.tensor.matmul(psB[:, j * 1024:j * 1024 + 512],
                                         w2t[0:64, j * 128:(j + 1) * 128],
                                         rb[0:64, :])
                        nc.tensor.matmul(psB[:, j * 1024 + 512:j * 1024 + 1024],
                                         w2t[64:128, j * 128:(j + 1) * 128],
                                         rb[64:128, :])
                    if use_softplus:
                        sout = e2s.tile([128, 2048], bf16, tag='sout')
                        nc.scalar.activation(sout[:, 0:w], psB[:, 0:w],
                                             AF.Softplus)
                        nc.sync.dma_start(sT[:, s0 * 1024:s0 * 1024 + w],
                                          sout[:, 0:w])
                    else:
                        nc.scalar.activation(sexp[:, s0 * 1024:s0 * 1024 + w],
                                             psB[:, 0:w], AF.Exp)
                if not use_softplus:
                    # gate: forces every Ln after the last Exp (Copy is in all
                    # ACT tables, so this adds no table switch)
                    gate = e2w.tile([128, 1], f32, tag='gate')
                    nc.scalar.activation(gate[:], sexp[:, nst_pc * 1024 - 1:nst_pc * 1024], AF.Copy,
                                         bias=1.0, scale=0.0)
                    ncols = nst_pc * 1024
                    step = 8192
                    for c0 in range(0, ncols, step):
                        w = min(step, ncols - c0)
                        sout = e2s.tile([128, step], bf16, tag='sout')
                        nc.scalar.activation(sout[:, 0:w], sexp[:, c0:c0 + w],
                                             AF.Ln, bias=1.0, scale=gate[:])
                        nc.sync.dma_start(sT[:, c0:c0 + w], sout[:, 0:w])

        # ---------- triplet phase ----------
        with (
            tc.tile_pool(name='t_w', bufs=1) as twp,
            tc.tile_pool(name='t_in', bufs=4) as tin,
            tc.tile_pool(name='t_sb', bufs=2) as tsb,
            tc.tile_pool(name='t_ue', bufs=1) as tue,
            tc.tile_pool(name='t_ps', bufs=2, space='PSUM') as tps,
        ):
            w3t = twp.tile([12, 128], bf16, tag='w3')
            nc.sync.dma_start(w3t[:], tw3[:])
            uexp = None
            if not use_softplus:
                uexp = tue.tile([128, nmt_pc * 512], bf16, tag='uexp')
            for b in range(n_ct):
                m0 = b * 4
                nm = min(4, nmt_pc - m0)
                w = nm * 512
                rows = tin.tile([12, 2048], bf16, tag='trows')
                nc.sync.dma_start(rows[:, 0:w],
                                  trows[:, m0 * 512:(m0 + nm) * 512])
                psC = tps.tile([128, 2048], f32, tag='psC')
                for j in range(nm):
                    nc.tensor.matmul(psC[:, j * 512:(j + 1) * 512],
                                     w3t[:], rows[:, j * 512:(j + 1) * 512])
                if use_softplus:
                    uout = tsb.tile([128, 2048], bf16, tag='uout')
                    nc.scalar.activation(uout[:, 0:w], psC[:, 0:w], AF.Softplus)
                    nc.sync.dma_start(uT[:, m0 * 512:(m0 + nm) * 512],
                                      uout[:, 0:w])
                else:
                    nc.scalar.activation(uexp[:, m0 * 512:m0 * 512 + w],
                                         psC[:, 0:w], AF.Exp)
            if not use_softplus:
                gate = twp.tile([128, 1], f32, tag='tgate')
                nc.scalar.activation(gate[:], uexp[:, nmt_pc * 512 - 1:nmt_pc * 512], AF.Copy,
                                     bias=1.0, scale=0.0)
                ncols = nmt_pc * 512
                step = 8192
                for c0 in range(0, ncols, step):
                    w = min(step, ncols - c0)
                    uout = tsb.tile([128, step], bf16, tag='uoutl')
                    nc.scalar.activation(uout[:, 0:w], uexp[:, c0:c0 + w],
                                         AF.Ln, bias=1.0, scale=gate[:])
                    nc.sync.dma_start(uT[:, c0:c0 + w], uout[:, 0:w])

    nc.compile()
    _CACHED[key] = nc
    return nc


def _segsum(vals, idx, nseg):
    order = np.argsort(idx, kind='stable')
    sv = vals[order]
    si = idx[order]
    counts = np.bincount(si, minlength=nseg)
    out = np.zeros((nseg, vals.shape[1]), np.float32)
    nz = np.flatnonzero(counts)
    if nz.size:
        starts = np.concatenate([[0], np.cumsum(counts)])[nz]
        out[nz] = np.add.reduceat(sv, starts, axis=0)
    return out


def kernel(features, neighbour_distances, neighbour_list, triplet_idxs,
           angles, r_ij, r_ik, W_pre, W2b1, W2b2, W3b1, W3b2, W_post):
    d_all = np.asarray(neighbour_distances, np.float32)
    nl = np.asarray(neighbour_list)
    t1 = np.asarray(triplet_idxs)[:, 1]
    W2b1 = np.asarray(W2b1, np.float32)
    W2b2 = np.asarray(W2b2, np.float32)
    W3b1 = np.asarray(W3b1, np.float32)
    W3b2 = np.asarray(W3b2, np.float32)
    centers = np.linspace(0.0, CUTOFF, E, dtype=np.float32)

    # ---------------- edge host prep ----------------
    keep = d_all < CUTOFF
    kept_idx = np.flatnonzero(keep)
    d = d_all[kept_idx]
    b = d / DC
    cls = np.where(b < CB0, 0, np.where(b < CB1, 1, 2)).astype(np.int32)
    order = np.argsort(cls, kind='stable')
    kept_sorted = kept_idx[order]
    d_s = d[order]
    ncls = np.bincount(cls[order], minlength=3)

    nst_cls = [(int(n) + ST_E - 1) // ST_E for n in ncls]
    nst_tot = sum(nst_cls)
    nst_tot_pad = ((nst_tot + NCORES - 1) // NCORES) * NCORES
    nst_pc = nst_tot_pad // NCORES
    n_epad = nst_tot_pad * ST_E

    dP = np.zeros(n_epad, np.float32)
    zz = np.full(n_epad, -30.0, np.float32)
    st_cls = np.zeros(nst_tot_pad, np.int32)
    src_pos = np.full(n_epad, -1, np.int64)
    off_e = off_p = st_i = 0
    for c in range(3):
        n = int(ncls[c])
        shift = (W0S[c] + 15.5) * DC
        dseg = d_s[off_e:off_e + n] - shift
        env = 0.5 * (1.0 + np.cos(np.pi * d_s[off_e:off_e + n] / CUTOFF))
        dP[off_p:off_p + n] = dseg
        zz[off_p:off_p + n] = (np.log(np.maximum(env, 1e-35))
                               - GAMMA * dseg * dseg)
        src_pos[off_p:off_p + n] = off_e + np.arange(n)
        st_cls[st_i:st_i + nst_cls[c]] = c
        off_e += n
        off_p += nst_cls[c] * ST_E
        st_i += nst_cls[c]

    d_hi, d_lo = _hilo(dP)
    z_hi, z_lo = _hilo(zz)
    ones = np.ones(n_epad, BF16)
    zero = np.zeros(n_epad, BF16)
    # rows per chunk: [d_hi, d_lo, d_hi, z_hi, z_lo, 1, 1, 0]
    rows8 = np.stack([d_hi, d_lo, d_hi, z_hi, z_lo, ones, ones, zero])
    rows32 = (rows8.reshape(8, nst_tot_pad, 4, 512)
              .transpose(2, 0, 1, 3).reshape(32, nst_tot_pad * 512))

    aw_cls = np.zeros((3, 32, 128), BF16)
    w2_cls = np.zeros((3, 128, 128), BF16)
    for c in range(3):
        w0 = W0S[c]
        cp = centers[w0:w0 + 32] - (w0 + 15.5) * DC
        a_hi, a_lo = _hilo(2.0 * GAMMA * cp)
        b_hi, b_lo = _hilo(-GAMMA * cp * cp)
        one32 = np.ones(32, BF16)
        zero32 = np.zeros(32, BF16)
        blk = np.stack([a_hi, a_hi, a_lo, one32, one32, b_hi, b_lo, zero32])
        for j in range(4):
            aw_cls[c, 8 * j:8 * j + 8, 32 * j:32 * j + 32] = blk
        wwin = W2b1[w0:w0 + 32, :].astype(BF16)
        w2_cls[c, 0:32, 0:64] = wwin
        w2_cls[c, 32:64, 64:128] = wwin
        w2_cls[c, 64:96, 0:64] = wwin
        w2_cls[c, 96:128, 64:128] = wwin

    # ---------------- triplet host prep ----------------
    nmt_tot = (N_TRIP + MT_T - 1) // MT_T
    nmt_tot = ((nmt_tot + NCORES - 1) // NCORES) * NCORES
    ntp = nmt_tot * MT_T
    nmt_pc = nmt_tot // NCORES

    rij = np.zeros(ntp, np.float32)
    rik = np.zeros(ntp, np.float32)
    cosa = np.zeros(ntp, np.float32)
    rij[:N_TRIP] = np.asarray(r_ij, np.float32)
    rik[:N_TRIP] = np.asarray(r_ik, np.float32)
    cosa[:N_TRIP] = np.cos(np.asarray(angles, np.float32))
    rij_h, rij_l = _hilo(rij)
    rik_h, rik_l = _hilo(rik)
    cos_h, cos_l = _hilo(cosa)
    rows6 = np.stack([rij_h, rij_l, rik_h, rik_l, cos_h, cos_l])
    rows12 = (rows6.reshape(6, nmt_tot, 2, 512)
              .transpose(2, 0, 1, 3).reshape(12, nmt_tot * 512))

    w3rows = W3b1.astype(BF16)
    w3dup = np.stack([w3rows[0], w3rows[0], w3rows[1], w3rows[1],
                      w3rows[2], w3rows[2]])
    tw3_np = np.zeros((12, 128), BF16)
    tw3_np[0:6, 0:64] = w3dup
    tw3_np[6:12, 64:128] = w3dup

    # ---------------- build + run ----------------
    nc = _build(nst_pc, nmt_pc)
    in_maps = []
    for k in range(NCORES):
        es = slice(k * nst_pc * 512, (k + 1) * nst_pc * 512)
        sts = slice(k * nst_pc, (k + 1) * nst_pc)
        ts = slice(k * nmt_pc * 512, (k + 1) * nmt_pc * 512)
        ccls = st_cls[sts]
        in_maps.append({
            'erows': np.ascontiguousarray(rows32[:, es]),
            'eaw': np.ascontiguousarray(aw_cls[ccls]),
            'ew2': np.ascontiguousarray(w2_cls[ccls]),
            'trows': np.ascontiguousarray(rows12[:, ts]),
            'tw3': tw3_np,
        })
    res = bass_utils.run_bass_kernel_spmd(nc, in_maps, core_ids=list(range(NCORES)))
    kernel.last_results = res

    sT = np.concatenate([np.asarray(r['sT'], BF16) for r in res.results],
                        axis=1).astype(np.float32)
    uT = np.concatenate([np.asarray(r['uT'], BF16) for r in res.results],
                        axis=1).astype(np.float32)

    s_pad = (sT.reshape(2, 64, nst_tot_pad, 2, 512)
             .transpose(1, 2, 3, 0, 4).reshape(64, n_epad))
    u_pad = (uT.reshape(2, 64, nmt_tot, 512)
             .transpose(1, 2, 0, 3).reshape(64, ntp))

    # ---------------- host combine ----------------
    h = np.asarray(features, np.float32) @ np.asarray(W_pre, np.float32)

    valid = src_pos >= 0
    s_sorted = np.empty((int(ncls.sum()), 64), np.float32)
    s_sorted[src_pos[valid]] = s_pad[:, valid].T
    m_kept = s_sorted @ W2b2
    m_kept += (-LOG2) * W2b2.sum(axis=0)
    nl0_k = nl[0][kept_sorted]
    nl1_k = nl[1][kept_sorted]
    two_body = h[nl1_k] * m_kept
    agg = _segsum(two_body, nl0_k, N_NODES)

    u = u_pad[:, :N_TRIP].T
    U3 = _segsum(u, t1, N_NODES)
    U3 -= LOG2 * np.bincount(t1, minlength=N_NODES)[:, None].astype(np.float32)
    em = h[:N_NODES] * (U3 @ W3b2)
    agg += _segsum(em, nl[0][:N_NODES], N_NODES)

    return (agg @ np.asarray(W_post, np.float32)).astype(np.float32)


# revision 18
# speedup vs baseline: 1.3620x; 1.0044x over previous
"""M3GNet interaction kernel for 8 Trainium2 NeuronCores.

Device computes the dense per-edge radial-basis MLP activations and the
per-triplet angular MLP activations; edges and triplets are sharded 8 ways
(graph/data parallel). Host does the index-based gathers/segment sums and
the small channel-mixing matmuls.

Device-side structure (per core):
- Edge radial basis exploits Gaussian locality: each edge only sees a
  32-center window of the 64 RBF centers (3 overlapping window classes,
  edges bucketed by distance on host, class resolved via per-supertile
  stationary weights streamed as data). The exp argument
  -gamma*(d-c)^2 + ln(env) comes from one K=32 block-diagonal matmul per
  2048-edge supertile (4 chunks of 512 packed into 128 partitions), with
  hi/lo bf16 operand splitting for fp32-grade accuracy at full PE rate
  (the per-center -gamma*c'^2 bias is folded in as hi/lo constant rows).
- Triplet path: one K=12 block-diagonal matmul per 1024 triplets.
- softplus: native Softplus activation, or a staged Exp->Ln(1+x) pipeline
  batched by activation function to avoid ACT table-set thrash.
Outputs stream back as bf16.
"""
import sys
import types

import numpy as np
import ml_dtypes

import concourse.bacc as bacc
import concourse.bass as bass
import concourse.mybir as mybir
from concourse.tile import TileContext
from concourse import bass_utils

try:  # pragma: no cover - environment shim
    import antenv.axon_hooks  # noqa: F401
except ImportError:
    # bass_utils imports antenv.axon_hooks unconditionally when BASS_TRACE=1
    # under axon; provide a no-op hook module so tracing degrades gracefully
    # instead of crashing in environments without it.
    try:
        import antenv
        _mod = types.ModuleType('antenv.axon_hooks')
        _mod._hook = None
        _mod.set_axon_ntff_profile_hook = lambda h: setattr(_mod, '_hook', h)
        _mod.get_axon_ntff_profile_hook = lambda: _mod._hook
        sys.modules['antenv.axon_hooks'] = _mod
        antenv.axon_hooks = _mod
    except Exception:
        pass

BF16 = ml_dtypes.bfloat16

N_NODES = 20000
N_EDGES = 640000
N_TRIP = 1000000
C = 128
E = 64
CUTOFF = 5.0
LOG2 = float(np.log(2.0))
NCORES = 8
DC = CUTOFF / (E - 1)                       # center spacing
GAMMA = 1.0 / (2.0 * (CUTOFF / E) ** 2)
W0S = (0, 16, 32)                           # window starts per class
CB0, CB1 = 21.5, 37.5                       # class boundaries in bin units

ST_E = 2048       # edges per supertile (4 chunks of 512)
MT_T = 1024       # triplets per matmul tile (2 chunks of 512)

USE_SOFTPLUS = False     # no softplus ACT table in this toolchain

_CACHED = {}


def _hilo(x):
    x = np.asarray(x, np.float32)
    hi = x.astype(BF16)
    lo = (x - hi.astype(np.float32)).astype(BF16)
    return hi, lo


def _build(nst_pc, nmt_pc, use_softplus=USE_SOFTPLUS):
    key = (nst_pc, nmt_pc, use_softplus)
    if key in _CACHED:
        return _CACHED[key]
    nc = bacc.Bacc('TRN2', target_bir_lowering=False, debug=False)
    f32 = mybir.dt.float32
    bf16 = mybir.dt.bfloat16
    AF = mybir.ActivationFunctionType

    erows = nc.dram_tensor('erows', [32, nst_pc * 512], bf16, kind='ExternalInput')
    eaw = nc.dram_tensor('eaw', [nst_pc, 32, 128], bf16, kind='ExternalInput')
    ew2 = nc.dram_tensor('ew2', [nst_pc, 128, 128], bf16, kind='ExternalInput')
    trows = nc.dram_tensor('trows', [12, nmt_pc * 512], bf16, kind='ExternalInput')
    tw3 = nc.dram_tensor('tw3', [12, 128], bf16, kind='ExternalInput')

    sT = nc.dram_tensor('sT', [128, nst_pc * 1024], bf16, kind='ExternalOutput')
    uT = nc.dram_tensor('uT', [128, nmt_pc * 512], bf16, kind='ExternalOutput')

    n_eg = (nst_pc + 3) // 4      # edge stage-1 groups of 4 supertiles
    n_bt = (nst_pc + 1) // 2      # edge stage-2 psB tiles (2 supertiles each)
    n_ct = (nmt_pc + 3) // 4      # triplet psC tiles (4 mm tiles each)

    with TileContext(nc) as tc:
        with tc.tile_pool(name='rbe_w', bufs=1) as rbw:
            warm = rbw.tile([128, 8], f32, tag='warm')
            nc.vector.tensor_scalar(warm[:], warm[:], 0.0, None,
                                    mybir.AluOpType.mult)
            nc.scalar.activation(warm[:], warm[:], AF.Exp)
            rbe = rbw.tile([128, nst_pc * 512], bf16, tag='rbe')

            # ---------- edge stage 1: arg matmuls + Exp -> rbe_wide ----------
            with (
                tc.tile_pool(name='e1_in', bufs=4) as e1i,
                tc.tile_pool(name='e1_w', bufs=4) as e1w,
                tc.tile_pool(name='e1_ps', bufs=2, space='PSUM') as ps1,
            ):
                for g in range(n_eg):
                    s0 = g * 4
                    ns = min(4, nst_pc - s0)
                    w = ns * 512
                    rows = e1i.tile([32, 2048], bf16, tag='erows')
                    nc.sync.dma_start(rows[:, 0:w],
                                      erows[:, s0 * 512:s0 * 512 + w])
                    awt = e1w.tile([32, 512], bf16, tag='aw')
                    nc.sync.dma_start(awt[:, 0:ns * 128],
                                      bass.AP(eaw, s0 * 32 * 128,
                                              [[128, 32], [4096, ns], [1, 128]]))
                    psA = ps1.tile([128, 2048], f32, tag='psA')
                    for j in range(ns):
                        nc.tensor.matmul(psA[:, j * 512:(j + 1) * 512],
                                         awt[:, j * 128:(j + 1) * 128],
                                         rows[:, j * 512:(j + 1) * 512])
                    nc.scalar.activation(rbe[:, s0 * 512:s0 * 512 + w],
                                         psA[:, 0:w], AF.Exp)

            # ---------- edge stage 2: p1 matmuls + softplus + out ----------
            with (
                tc.tile_pool(name='e2_w', bufs=3) as e2w,
                tc.tile_pool(name='e2_sb', bufs=2) as e2s,
                tc.tile_pool(name='e2_se', bufs=1) as e2e,
                tc.tile_pool(name='e2_ps', bufs=2, space='PSUM') as ps2,
            ):
                sexp = None
                if not use_softplus:
                    sexp = e2e.tile([128, nst_pc * 1024], bf16, tag='sexp')
                for b in range(n_bt):
                    s0 = b * 2
                    ns = min(2, nst_pc - s0)
                    w = ns * 1024
                    w2t = e2w.tile([128, 256], bf16, tag='w2')
                    nc.sync.dma_start(w2t[:, 0:ns * 128],
                                      bass.AP(ew2, s0 * 128 * 128,
                                              [[128, 128], [16384, ns], [1, 128]]))
                    psB = ps2.tile([128, 2048], f32, tag='psB')
                    for j in range(ns):
                        s = s0 + j
                        rb = rbe[:, s * 512:(s + 1) * 512]
                        nc.tensor.matmul(psB[:, j * 1024:j * 1024 + 512],
                                         w2t[0:64, j * 128:(j + 1) * 128],
                                         rb[0:64, :])
                        nc.tensor.matmul(psB[:, j * 1024 + 512:j * 1024 + 1024],
                                         w2t[64:128, j * 128:(j + 1) * 128],
                                         rb[64:128, :])
                    if use_softplus:
                        sout = e2s.tile([128, 2048], bf16, tag='sout')
                        nc.scalar.activation(sout[:, 0:w], psB[:, 0:w],
                                             AF.Softplus)
                        nc.sync.dma_start(sT[:, s0 * 1024:s0 * 1024 + w],
                                          sout[:, 0:w])
                    else:
                        nc.scalar.activation(sexp[:, s0 * 1024:s0 * 1024 + w],
                                             psB[:, 0:w], AF.Exp)
                if not use_softplus:
                    # gate: forces every Ln after the last Exp (Copy is in all
                    # ACT tables, so this adds no table switch)
                    gate = e2w.tile([128, 1], f32, tag='gate')
                    nc.scalar.activation(gate[:], sexp[:, nst_pc * 1024 - 1:nst_pc * 1024], AF.Copy,
                                         bias=1.0, scale=0.0)
                    ncols = nst_pc * 1024
                    step = 8192
                    for c0 in range(0, ncols, step):
                        w = min(step, ncols - c0)
                        sout = e2s.tile([128, step], bf16, tag='sout')
                        nc.scalar.activation(sout[:, 0:w], sexp[:, c0:c0 + w],
                                             AF.Ln, bias=1.0, scale=gate[:])
                        nc.sync.dma_start(sT[:, c0:c0 + w], sout[:, 0:w])

        # ---------- triplet phase ----------
        with (
            tc.tile_pool(name='t_w', bufs=1) as twp,
            tc.tile_pool(name='t_in', bufs=4) as tin,
            tc.tile_pool(name='t_sb', bufs=2) as tsb,
            tc.tile_pool(name='t_ue', bufs=1) as tue,
            tc.tile_pool(name='t_ps', bufs=2, space='PSUM') as tps,
        ):
            w3t = twp.tile([12, 128], bf16, tag='w3')
            nc.sync.dma_start(w3t[:], tw3[:])
            uexp = None
            if not use_softplus:
                uexp = tue.tile([128, nmt_pc * 512], bf16, tag='uexp')
            for b in range(n_ct):
                m0 = b * 4
                nm = min(4, nmt_pc - m0)
                w = nm * 512
                rows = tin.tile([12, 2048], bf16, tag='trows')
                nc.sync.dma_start(rows[:, 0:w],
                                  trows[:, m0 * 512:(m0 + nm) * 512])
                psC = tps.tile([128, 2048], f32, tag='psC')
                for j in range(nm):
                    nc.tensor.matmul(psC[:, j * 512:(j + 1) * 512],
                                     w3t[:], rows[:, j * 512:(j + 1) * 512])
                if use_softplus:
                    uout = tsb.tile([128, 2048], bf16, tag='uout')
                    nc.scalar.activation(uout[:, 0:w], psC[:, 0:w], AF.Softplus)
                    nc.sync.dma_start(uT[:, m0 * 512:(m0 + nm) * 512],
                                      uout[:, 0:w])
                else:
                    nc.scalar.activation(uexp[:, m0 * 512:m0 * 512 + w],
                                         psC[:, 0:w], AF.Exp)
            if not use_softplus:
                gate = twp.tile([128, 1], f32, tag='tgate')
                nc.scalar.activation(gate[:], uexp[:, nmt_pc * 512 - 1:nmt_pc * 512], AF.Copy,
                                     bias=1.0, scale=0.0)
                ncols = nmt_pc * 512
                step = 8192
                for c0 in range(0, ncols, step):
                    w = min(step, ncols - c0)
                    uout = tsb.tile([128, step], bf16, tag='uoutl')
                    nc.scalar.activation(uout[:, 0:w], uexp[:, c0:c0 + w],
                                         AF.Ln, bias=1.0, scale=gate[:])
                    nc.sync.dma_start(uT[:, c0:c0 + w], uout[:, 0:w])

    nc.compile()
    _CACHED[key] = nc
    return nc


def _segsum(vals, idx, nseg):
    order = np.argsort(idx, kind='stable')
    sv = vals[order]
    si = idx[order]
    counts = np.bincount(si, minlength=nseg)
    out = np.zeros((nseg, vals.shape[1]), np.float32)
    nz = np.flatnonzero(counts)
    if nz.size:
        starts = np.concatenate([[0], np.cumsum(counts)])[nz]
        out[nz] = np.add.reduceat(sv, starts, axis=0)
    return out


def kernel(features, neighbour_distances, neighbour_list, triplet_idxs,
           angles, r_ij, r_ik, W_pre, W2b1, W2b2, W3b1, W3b2, W_post):
    d_all = np.asarray(neighbour_distances, np.float32)
    nl = np.asarray(neighbour_list)
    t1 = np.asarray(triplet_idxs)[:, 1]
    W2b1 = np.asarray(W2b1, np.float32)
    W2b2 = np.asarray(W2b2, np.float32)
    W3b1 = np.asarray(W3b1, np.float32)
    W3b2 = np.asarray(W3b2, np.float32)
    centers = np.linspace(0.0, CUTOFF, E, dtype=np.float32)

    # ---------------- edge host prep ----------------
    keep = d_all < CUTOFF
    kept_idx = np.flatnonzero(keep)
    d = d_all[kept_idx]
    b = d / DC
    cls = np.where(b < CB0, 0, np.where(b < CB1, 1, 2)).astype(np.int32)
    order = np.argsort(cls, kind='stable')
    kept_sorted = kept_idx[order]
    d_s = d[order]
    ncls = np.bincount(cls[order], minlength=3)

    nst_cls = [(int(n) + ST_E - 1) // ST_E for n in ncls]
    nst_tot = sum(nst_cls)
    nst_tot_pad = ((nst_tot + NCORES - 1) // NCORES) * NCORES
    nst_pc = nst_tot_pad // NCORES
    n_epad = nst_tot_pad * ST_E

    dP = np.zeros(n_epad, np.float32)
    zz = np.full(n_epad, -30.0, np.float32)
    st_cls = np.zeros(nst_tot_pad, np.int32)
    src_pos = np.full(n_epad, -1, np.int64)
    off_e = off_p = st_i = 0
    for c in range(3):
        n = int(ncls[c])
        shift = (W0S[c] + 15.5) * DC
        dseg = d_s[off_e:off_e + n] - shift
        env = 0.5 * (1.0 + np.cos(np.pi * d_s[off_e:off_e + n] / CUTOFF))
        dP[off_p:off_p + n] = dseg
        zz[off_p:off_p + n] = (np.log(np.maximum(env, 1e-35))
                               - GAMMA * dseg * dseg)
        src_pos[off_p:off_p + n] = off_e + np.arange(n)
        st_cls[st_i:st_i + nst_cls[c]] = c
        off_e += n
        off_p += nst_cls[c] * ST_E
        st_i += nst_cls[c]

    d_hi, d_lo = _hilo(dP)
    z_hi, z_lo = _hilo(zz)
    ones = np.ones(n_epad, BF16)
    zero = np.zeros(n_epad, BF16)
    # rows per chunk: [d_hi, d_lo, d_hi, z_hi, z_lo, 1, 1, 0]
    rows8 = np.stack([d_hi, d_lo, d_hi, z_hi, z_lo, ones, ones, zero])
    rows32 = (rows8.reshape(8, nst_tot_pad, 4, 512)
              .transpose(2, 0, 1, 3).reshape(32, nst_tot_pad * 512))

    aw_cls = np.zeros((3, 32, 128), BF16)
    w2_cls = np.zeros((3, 128, 128), BF16)
    for c in range(3):
        w0 = W0S[c]
        cp = centers[w0:w0 + 32] - (w0 + 15.5) * DC
        a_hi, a_lo = _hilo(2.0 * GAMMA * cp)
        b_hi, b_lo = _hilo(-GAMMA * cp * cp)
        one32 = np.ones(32, BF16)
        zero32 = np.zeros(32, BF16)
        blk = np.stack([a_hi, a_hi, a_lo, one32, one32, b_hi, b_lo, zero32])
        for j in range(4):
            aw_cls[c, 8 * j:8 * j + 8, 32 * j:32 * j + 32] = blk
        wwin = W2b1[w0:w0 + 32, :].astype(BF16)
        w2_cls[c, 0:32, 0:64] = wwin
        w2_cls[c, 32:64, 64:128] = wwin
        w2_cls[c, 64:96, 0:64] = wwin
        w2_cls[c, 96:128, 64:128] = wwin

    # ---------------- triplet host prep ----------------
    nmt_tot = (N_TRIP + MT_T - 1) // MT_T
    nmt_tot = ((nmt_tot + NCORES - 1) // NCORES) * NCORES
    ntp = nmt_tot * MT_T
    nmt_pc = nmt_tot // NCORES

    rij = np.zeros(ntp, np.float32)
    rik = np.zeros(ntp, np.float32)
    cosa = np.zeros(ntp, np.float32)
    rij[:N_TRIP] = np.asarray(r_ij, np.float32)
    rik[:N_TRIP] = np.asarray(r_ik, np.float32)
    cosa[:N_TRIP] = np.cos(np.asarray(angles, np.float32))
    rij_h, rij_l = _hilo(rij)
    rik_h, rik_l = _hilo(rik)
    cos_h, cos_l = _hilo(cosa)
    rows6 = np.stack([rij_h, rij_l, rik_h, rik_l, cos_h, cos_l])
    rows12 = (rows6.reshape(6, nmt_tot, 2, 512)
              .transpose(2, 0, 1, 3).reshape(12, nmt_tot * 512))

    w3rows = W3b1.astype(BF16)
    w3dup = np.stack([w3rows[0], w3rows[0], w3rows[1], w3rows[1],
                      w3rows[2], w3rows[2]])
    tw3_np = np.zeros((12, 128), BF16)
    tw3_np[0:6, 0:64] = w3dup
    tw3_np[6:12, 64:128] = w3dup

    # ---------------- build + run ----------------
    nc = _build(nst_pc, nmt_pc)
    in_maps = []
    for k in range(NCORES):
        es = slice(k * nst_pc * 512, (k + 1) * nst_pc * 512)
        sts = slice(k * nst_pc, (k + 1) * nst_pc)
        ts = slice(k * nmt_pc * 512, (k + 1) * nmt_pc * 512)
        ccls = st_cls[sts]
        in_maps.append({
            'erows': np.ascontiguousarray(rows32[:, es]),
            'eaw': np.ascontiguousarray(aw_cls[ccls]),
            'ew2': np.ascontiguousarray(w2_cls[ccls]),
            'trows': np.ascontiguousarray(rows12[:, ts]),
            'tw3': tw3_np,
        })
    res = bass_utils.run_bass_kernel_spmd(nc, in_maps, core_ids=list(range(NCORES)))
    kernel.last_results = res

    sT = np.concatenate([np.asarray(r['sT'], BF16) for r in res.results],
                        axis=1).astype(np.float32)
    uT = np.concatenate([np.asarray(r['uT'], BF16) for r in res.results],
                        axis=1).astype(np.float32)

    s_pad = (sT.reshape(2, 64, nst_tot_pad, 2, 512)
             .transpose(1, 2, 3, 0, 4).reshape(64, n_epad))
    u_pad = (uT.reshape(2, 64, nmt_tot, 512)
             .transpose(1, 2, 0, 3).reshape(64, ntp))

    # ---------------- host combine ----------------
    h = np.asarray(features, np.float32) @ np.asarray(W_pre, np.float32)

    valid = src_pos >= 0
    s_sorted = np.empty((int(ncls.sum()), 64), np.float32)
    s_sorted[src_pos[valid]] = s_pad[:, valid].T
    m_kept = s_sorted @ W2b2
    m_kept += (-LOG2) * W2b2.sum(axis=0)
    nl0_k = nl[0][kept_sorted]
    nl1_k = nl[1][kept_sorted]
    two_body = h[nl1_k] * m_kept
    agg = _segsum(two_body, nl0_k, N_NODES)

    u = u_pad[:, :N_TRIP].T
    U3 = _segsum(u, t1, N_NODES)
    U3 -= LOG2 * np.bincount(t1, minlength=N_NODES)[:, None].astype(np.float32)
    em = h[:N_NODES] * (U3 @ W3b2)
    agg += _segsum(em, nl[0][:N_NODES], N_NODES)

    return (agg @ np.asarray(W_post, np.float32)).astype(np.float32)
